# revision 26
# baseline (speedup 1.0000x reference)
"""Mamba MixerModel Trainium2 kernel.

Sharding: data-parallel over batch (8 cores x 1 batch element). No collectives.
Layout: d_inner on partitions for conv/scan; tokens on partitions for LN/residual.
Scan: native DVE tensor_tensor_scan (state = a*state + b) per (n, d-tile) strip,
with fp32 decays a = exp(A*dt) fused on ScalarE (per-partition scale), bf16
injections/outputs, sequence processed in quarters with bf16 carries.
"""
import sys, os
sys.path.insert(0, "/opt/trn_rl_repo")

import numpy as np
import ml_dtypes

import concourse.bass as bass
import concourse.bacc as bacc
import concourse.mybir as mybir
from concourse.tile import TileContext

F32 = mybir.dt.float32
BF16 = mybir.dt.bfloat16
AF = mybir.ActivationFunctionType
OP = mybir.AluOpType

B_, L_, C_IN, D_MODEL, N_LAYER = 8, 4096, 4, 256, 4
D_INNER, D_STATE, D_CONV, DT_RANK = 512, 16, 4, 16
EPS = 1e-5
NDT = D_INNER // 128  # 4 d-tiles
NMT = 2 * D_INNER // 128  # 8 xz row tiles
KDM = D_MODEL // 128  # 2 k-tiles over d_model


def build_nc(L=4096, TC=1024, ln_affine=True, debug_probe=False):
    NH = L // TC           # quarters
    NSUB = TC // 512 if TC >= 512 else 1   # psum subchunks per quarter
    SUB = min(512, TC)
    N128 = TC // 128       # 128-token chunks per quarter
    NCH = L // 128         # total 128-token chunks

    nc = bacc.Bacc(None, target_bir_lowering=False)
    dram = {}
    def din(name, shape, dt=F32):
        dram[name] = nc.dram_tensor(name, shape, dt, kind="ExternalInput")
        return dram[name]

    x_in = din("x_in", [C_IN, L])
    ident = din("ident", [128, 128])
    ident_bf = din("ident_bf", [128, 128], BF16)
    ones1 = din("ones1", [1, 128])
    emb_wT = din("emb_wT", [C_IN, D_MODEL])
    emb_b1 = din("emb_b1", [1, D_MODEL])
    in_wT = din("in_wT", [N_LAYER, KDM, 128, 2 * D_INNER])
    conv_w = din("conv_w", [N_LAYER, NDT, 128, D_CONV])
    conv_diag = din("conv_diag", [N_LAYER, NDT, D_CONV, 128, 128], BF16)
    conv_b = din("conv_b", [N_LAYER, NDT, 128, 1])
    xpwT = din("xpwT", [N_LAYER, NDT, 128, DT_RANK + 2 * D_STATE], BF16)
    dtwT = din("dtwT", [N_LAYER, DT_RANK, D_INNER], BF16)
    dtb = din("dtb", [N_LAYER, NDT, 128, 1])
    A_in = din("A_in", [N_LAYER, NDT, 128, D_STATE])
    Dsk = din("Dsk", [N_LAYER, NDT, 128, 1])
    owT = din("owT", [N_LAYER, NDT, 128, D_MODEL], BF16)
    nw = din("nw", [N_LAYER, 1, D_MODEL])
    nb = din("nb", [N_LAYER, 1, D_MODEL])
    nfw = din("nfw", [1, D_MODEL])
    nfb = din("nfb", [1, D_MODEL])
    out_y = nc.dram_tensor("out_y", [L, D_MODEL], F32, kind="ExternalOutput")
    if debug_probe:
        dbg_emb = nc.dram_tensor("dbg_emb", [128, (L // 128) * D_MODEL], F32, kind="ExternalOutput")
        dbg_hT = nc.dram_tensor("dbg_hT", [128, 2 * TC], F32, kind="ExternalOutput")
        dbg_xs = nc.dram_tensor("dbg_xs", [128, TC], BF16, kind="ExternalOutput")
        dbg_dt = nc.dram_tensor("dbg_dt", [128, TC], BF16, kind="ExternalOutput")
        dbg_dbl = nc.dram_tensor("dbg_dbl", [48, TC], BF16, kind="ExternalOutput")
        dbg_y = nc.dram_tensor("dbg_y", [128, TC], BF16, kind="ExternalOutput")
        dbg_l = [nc.dram_tensor(f"dbg_l{i}", [128, (L // 128) * D_MODEL], F32,
                                kind="ExternalOutput") for i in range(N_LAYER)]

    NX = DT_RANK + 2 * D_STATE  # 48

    with TileContext(nc) as tc:
        with (
            tc.tile_pool(name="const", bufs=1) as cpool,
            tc.tile_pool(name="wts", bufs=1) as wpool,
            tc.tile_pool(name="hres", bufs=1) as hpool,
            tc.tile_pool(name="big", bufs=2) as big,     # rotating big transients
            tc.tile_pool(name="one", bufs=1) as one,     # per-quarter single-buffered
            tc.tile_pool(name="strip", bufs=3) as sp,    # scan strips
            tc.tile_pool(name="small", bufs=2) as sm,
            tc.tile_pool(name="psum", bufs=4, space="PSUM") as ps,
            tc.tile_pool(name="psumy", bufs=4, space="PSUM") as psy,
            tc.tile_pool(name="dram", bufs=1, space="DRAM") as dpool,
        ):
            # ---- constants / global tiles
            id_t = cpool.tile([128, 128], F32)
            nc.sync.dma_start(out=id_t[:], in_=ident[:])
            idb_t = cpool.tile([128, 128], BF16)
            nc.sync.dma_start(out=idb_t[:], in_=ident_bf[:])
            ones_t = cpool.tile([1, 128], F32)
            nc.sync.dma_start(out=ones_t[:], in_=ones1[:])
            embw_t = cpool.tile([C_IN, D_MODEL], F32)
            nc.sync.dma_start(out=embw_t[:], in_=emb_wT[:])
            embb_t = cpool.tile([1, D_MODEL], F32)
            nc.sync.dma_start(out=embb_t[:], in_=emb_b1[:])
            h_res = hpool.tile([128, NCH * D_MODEL], F32)  # [t-chunk-major, dm]
            bc_stage = dpool.tile([2 * D_STATE, L], BF16)

            # ---- embedding: h_res = x @ emb_wT + emb_b
            for c in range(NCH):
                xc = sm.tile([C_IN, 128], F32, tag="xchunk")
                nc.sync.dma_start(out=xc[:], in_=x_in[:, c * 128:(c + 1) * 128])
                pt = ps.tile([128, D_MODEL], F32, tag="ps")
                nc.tensor.matmul(pt[:], xc[:], embw_t[:],
                                 start=True, stop=False)
                nc.tensor.matmul(pt[:], ones_t[:], embb_t[:], start=False, stop=True)
                nc.scalar.copy(h_res[:, c * D_MODEL:(c + 1) * D_MODEL], pt[:])
            if debug_probe:
                nc.sync.dma_start(out=dbg_emb[:], in_=h_res[:])

            def layernorm(widx, wt, bt, affine, dst_chunks):
                """LN over h_res; dst_chunks(c, tile[128, D_MODEL]) consumes output."""
                for q in range(NH):
                    c0 = q * N128
                    s1 = sm.tile([128, N128], F32, tag="st1")
                    src3 = h_res[:, c0 * D_MODEL:(c0 + N128) * D_MODEL].rearrange(
                        "p (c d) -> p c d", c=N128)
                    nc.vector.tensor_reduce(s1[:], src3, axis=mybir.AxisListType.X, op=OP.add)
                    s2 = sm.tile([128, N128], F32, tag="st2")
                    NHF = N128 // 2
                    for hh in range(2):
                        sq = one.tile([128, NHF * D_MODEL], F32, tag="sq")
                        nc.scalar.square(sq[:], h_res[:, (c0 + hh * NHF) * D_MODEL:
                                                      (c0 + (hh + 1) * NHF) * D_MODEL])
                        nc.vector.tensor_reduce(s2[:, hh * NHF:(hh + 1) * NHF],
                                                sq[:].rearrange("p (c d) -> p c d", c=NHF),
                                                axis=mybir.AxisListType.X, op=OP.add)
                    mean = sm.tile([128, N128], F32, tag="st3")
                    nc.scalar.mul(mean[:], s1[:], 1.0 / D_MODEL)
                    ex2 = sm.tile([128, N128], F32, tag="st4")
                    nc.scalar.mul(ex2[:], s2[:], 1.0 / D_MODEL)
                    var = sm.tile([128, N128], F32, tag="st5")
                    nc.vector.scalar_tensor_tensor(var[:], mean[:], -1.0, mean[:], OP.mult, OP.mult)
                    nc.vector.tensor_tensor(var[:], ex2[:], var[:], OP.add)
                    sq2 = sm.tile([128, N128], F32, tag="st6")
                    nc.vector.tensor_scalar(var[:], var[:], EPS, None, OP.add)
                    nc.scalar.activation(sq2[:], var[:], AF.Ln)
                    rstd = sm.tile([128, N128], F32, tag="st7")
                    nc.scalar.activation(rstd[:], sq2[:], AF.Exp, scale=-0.5)
                    negmr = sm.tile([128, N128], F32, tag="st8")
                    nc.vector.scalar_tensor_tensor(negmr[:], mean[:], -1.0, rstd[:], OP.mult, OP.mult)
                    for cc in range(N128):
                        c = c0 + cc
                        lt = sm.tile([128, D_MODEL], F32, tag="lnout")
                        nc.scalar.activation(lt[:], h_res[:, c * D_MODEL:(c + 1) * D_MODEL],
                                             AF.Identity, bias=negmr[:, cc:cc + 1],
                                             scale=rstd[:, cc:cc + 1])
                        if affine:
                            nc.vector.tensor_tensor(lt[:], lt[:], wt[:], OP.mult)
                            nc.vector.tensor_tensor(lt[:], lt[:], bt[:], OP.add)
                        dst_chunks(c, lt)

            # ================= layers =================
            for li in range(N_LAYER):
                # ---- load weights for this layer
                inw_t = wpool.tile([128, KDM * 2 * D_INNER], F32, tag="w_in")
                for k in range(KDM):
                    nc.sync.dma_start(out=inw_t[:, k * 2 * D_INNER:(k + 1) * 2 * D_INNER],
                                      in_=in_wT[li, k])
                cw_t = wpool.tile([128, NDT * D_CONV], F32, tag="w_cw")
                cb_t = wpool.tile([128, NDT], F32, tag="w_cb")
                xpw_t = wpool.tile([128, NDT * NX], BF16, tag="w_xp")
                dtw_t = wpool.tile([DT_RANK, D_INNER], BF16, tag="w_dtw")
                dtb_t = wpool.tile([128, NDT], F32, tag="w_dtb")
                A_t = wpool.tile([128, NDT * D_STATE], F32, tag="w_A")
                dsk_t = wpool.tile([128, NDT], F32, tag="w_dsk")
                ow_t = wpool.tile([128, NDT * D_MODEL], BF16, tag="w_ow")
                for d in range(NDT):
                    nc.sync.dma_start(out=cw_t[:, d * D_CONV:(d + 1) * D_CONV], in_=conv_w[li, d])
                    nc.sync.dma_start(out=cb_t[:, d:d + 1], in_=conv_b[li, d])
                    nc.sync.dma_start(out=xpw_t[:, d * NX:(d + 1) * NX], in_=xpwT[li, d])
                    nc.sync.dma_start(out=dtb_t[:, d:d + 1], in_=dtb[li, d])
                    nc.sync.dma_start(out=A_t[:, d * D_STATE:(d + 1) * D_STATE], in_=A_in[li, d])
                    nc.sync.dma_start(out=dsk_t[:, d:d + 1], in_=Dsk[li, d])
                    nc.sync.dma_start(out=ow_t[:, d * D_MODEL:(d + 1) * D_MODEL], in_=owT[li, d])
                nc.sync.dma_start(out=dtw_t[:], in_=dtwT[li])
                if ln_affine:
                    nw_t = wpool.tile([128, D_MODEL], F32, tag="w_nw")
                    nb_t = wpool.tile([128, D_MODEL], F32, tag="w_nb")
                    nc.sync.dma_start(out=nw_t[:], in_=nw[li].partition_broadcast(128))
                    nc.sync.dma_start(out=nb_t[:], in_=nb[li].partition_broadcast(128))
                else:
                    nw_t = nb_t = None

                carry = wpool.tile([128, D_STATE * NDT], BF16, tag="carry")
                cdg_t = wpool.tile([128, NDT * D_CONV * 128], BF16, tag="w_cdg")
                for d in range(NDT):
                    for k in range(D_CONV):
                        nc.sync.dma_start(
                            out=cdg_t[:, (d * D_CONV + k) * 128:(d * D_CONV + k + 1) * 128],
                            in_=conv_diag[li, d, k])
                def cdg(d, k):
                    return cdg_t[:, (d * D_CONV + k) * 128:(d * D_CONV + k + 1) * 128]
                cbh_t = wpool.tile([128, NDT], F32, tag="w_cbh")
                nc.vector.tensor_scalar(cbh_t[:], cb_t[:], 0.5, None, OP.mult)

                # per-quarter state (python lists of tiles)
                xbf_prev = [None] * NDT  # for conv halo

                # LN output -> transpose -> hT per quarter, so capture per-quarter tiles
                hT_tiles = {}

                def make_dst(hT_tiles):
                    def dst(c, lt):
                        q = c // N128
                        cc = c % N128
                        hT = hT_tiles.get(q)
                        if hT is None:
                            hT = big.tile([128, KDM * TC], F32, tag="hT")
                            hT_tiles[q] = hT
                        for km in range(KDM):
                            pt = ps.tile([128, 128], F32, tag="ps")
                            nc.tensor.transpose(pt[:], lt[:, km * 128:(km + 1) * 128], id_t[:])
                            nc.scalar.copy(hT[:, km * TC + cc * 128: km * TC + (cc + 1) * 128], pt[:])
                    return dst

                # run LN for whole sequence quarter by quarter, but we need hT per
                # quarter before the rest of that quarter's pipeline. Do it inline:
                # process quarters fully sequentially below instead.
                for q in range(NH):
                    # --- LN for this quarter's chunks + transpose into hT
                    hT_tiles_q = {}
                    dstf = make_dst(hT_tiles_q)
                    c0 = q * N128
                    # (inline layernorm for quarter q)
                    s1 = sm.tile([128, N128], F32, tag="st1")
                    src3 = h_res[:, c0 * D_MODEL:(c0 + N128) * D_MODEL].rearrange(
                        "p (c d) -> p c d", c=N128)
                    nc.vector.tensor_reduce(s1[:], src3, axis=mybir.AxisListType.X, op=OP.add)
                    s2 = sm.tile([128, N128], F32, tag="st2")
                    NHF = N128 // 2
                    for hh in range(2):
                        sq = one.tile([128, NHF * D_MODEL], F32, tag="sq")
                        nc.scalar.square(sq[:], h_res[:, (c0 + hh * NHF) * D_MODEL:
                                                      (c0 + (hh + 1) * NHF) * D_MODEL])
                        nc.vector.tensor_reduce(s2[:, hh * NHF:(hh + 1) * NHF],
                                                sq[:].rearrange("p (c d) -> p c d", c=NHF),
                                                axis=mybir.AxisListType.X, op=OP.add)
                    mean = sm.tile([128, N128], F32, tag="st3")
                    nc.scalar.mul(mean[:], s1[:], 1.0 / D_MODEL)
                    ex2 = sm.tile([128, N128], F32, tag="st4")
                    nc.scalar.mul(ex2[:], s2[:], 1.0 / D_MODEL)
                    var = sm.tile([128, N128], F32, tag="st5")
                    nc.vector.scalar_tensor_tensor(var[:], mean[:], -1.0, mean[:], OP.mult, OP.mult)
                    nc.vector.tensor_tensor(var[:], ex2[:], var[:], OP.add)
                    sq2 = sm.tile([128, N128], F32, tag="st6")
                    nc.vector.tensor_scalar(var[:], var[:], EPS, None, OP.add)
                    nc.scalar.activation(sq2[:], var[:], AF.Ln)
                    rstd = sm.tile([128, N128], F32, tag="st7")
                    nc.scalar.activation(rstd[:], sq2[:], AF.Exp, scale=-0.5)
                    negmr = sm.tile([128, N128], F32, tag="st8")
                    nc.vector.scalar_tensor_tensor(negmr[:], mean[:], -1.0, rstd[:], OP.mult, OP.mult)
                    for cc in range(N128):
                        c = c0 + cc
                        lt = sm.tile([128, D_MODEL], F32, tag="lnout")
                        nc.scalar.activation(lt[:], h_res[:, c * D_MODEL:(c + 1) * D_MODEL],
                                             AF.Identity, bias=negmr[:, cc:cc + 1],
                                             scale=rstd[:, cc:cc + 1])
                        if ln_affine:
                            nc.vector.tensor_tensor(lt[:], lt[:], nw_t[:], OP.mult)
                            nc.vector.tensor_tensor(lt[:], lt[:], nb_t[:], OP.add)
                        dstf(c, lt)
                    hT = hT_tiles_q[q]

                    if debug_probe and li == 0 and q == 0:
                        nc.sync.dma_start(out=dbg_hT[:], in_=hT[:])
                    # --- in_proj -> x (bf16, per d-tile) and g = silu(z)
                    x_bf = [big.tile([128, TC], BF16, tag=f"xbf{d}", name=f"xbf{d}") for d in range(NDT)]
                    g_bf = [one.tile([128, TC], BF16, tag=f"gbf{d}", name=f"gbf{d}") for d in range(NDT)]
                    z_bf = [one.tile([128, TC], BF16, tag=f"zbf{d}", name=f"zbf{d}") for d in range(NDT)]
                    for m in range(NMT):  # 8 row-tiles of xz
                        for s in range(NSUB):
                            pt = ps.tile([128, SUB], F32, tag="ps")
                            for k in range(KDM):
                                nc.tensor.matmul(
                                    pt[:],
                                    inw_t[:, k * 2 * D_INNER + m * 128:
                                          k * 2 * D_INNER + (m + 1) * 128],
                                    hT[:, k * TC + s * SUB: k * TC + (s + 1) * SUB],
                                    start=(k == 0), stop=(k == KDM - 1))
                            if m < NDT:
                                nc.scalar.copy(x_bf[m][:, s * SUB:(s + 1) * SUB], pt[:])
                            else:
                                d = m - NDT
                                nc.scalar.copy(z_bf[d][:, s * SUB:(s + 1) * SUB], pt[:])
                                nc.scalar.activation(g_bf[d][:, s * SUB:(s + 1) * SUB],
                                                     pt[:], AF.Tanh, scale=0.5)

                    # --- conv + silu -> xs ; dtx ; y0
                    xs_bf = [one.tile([128, TC], BF16, tag=f"xsbf{d}", name=f"xsbf{d}") for d in range(NDT)]
                    for d in range(NDT):
                        th = big.tile([128, TC], BF16, tag="thbuf")
                        ub = big.tile([128, TC], BF16, tag="ubuf")
                        for s in range(NSUB):
                            base = s * SUB
                            cps = ps.tile([128, SUB], F32, tag="ps")
                            # collect (out_ap, rhs_ap, k) then emit with flags
                            mms = [(cps[:, 0:SUB], x_bf[d][:, base:base + SUB], 3)]
                            for k in range(3):
                                sh = 3 - k
                                if base - sh >= 0:
                                    mms.append((cps[:, 0:SUB],
                                                x_bf[d][:, base - sh:base + SUB - sh], k))
                                else:
                                    mms.append((cps[:, sh:SUB],
                                                x_bf[d][:, 0:base + SUB - sh], k))
                                    if q > 0:
                                        mms.append((cps[:, 0:sh],
                                                    xbf_prev[d][:, TC - sh:TC], k))
                            for i, (oap, rap, k) in enumerate(mms):
                                nc.tensor.matmul(oap, cdg(d, k), rap,
                                                 start=(i == 0), stop=(i == len(mms) - 1),
                                                 skip_group_check=True)
                            nc.scalar.activation(th[:, base:base + SUB], cps[:], AF.Tanh,
                                                 scale=0.5, bias=cbh_t[:, d:d + 1])
                            nc.scalar.activation(ub[:, base:base + SUB], cps[:], AF.Identity,
                                                 bias=cb_t[:, d:d + 1])
                        nc.vector.tensor_scalar(th[:], th[:], 0.5, 0.5, OP.mult, OP.add)
                        nc.vector.tensor_tensor(xs_bf[d][:], ub[:], th[:], OP.mult)
                    xbf_prev = x_bf
                    if debug_probe and li == 0 and q == 0:
                        nc.sync.dma_start(out=dbg_xs[:], in_=xs_bf[0][:])

                    # --- x_proj -> dbl (48 rows)
                    dbl = one.tile([NX, TC], BF16, tag="dbl")
                    for s in range(NSUB):
                        pt = ps.tile([NX, SUB], F32, tag="ps")
                        for d in range(NDT):
                            nc.tensor.matmul(pt[:], xpw_t[:, d * NX:(d + 1) * NX],
                                             xs_bf[d][:, s * SUB:(s + 1) * SUB],
                                             start=(d == 0), stop=(d == NDT - 1))
                        nc.scalar.copy(dbl[:, s * SUB:(s + 1) * SUB], pt[:])
                    if debug_probe and li == 0 and q == 0:
                        nc.sync.dma_start(out=dbg_dbl[:], in_=dbl[:])
                    # stage B/C rows to DRAM for partition-broadcast
                    nc.sync.dma_start(out=bc_stage[:, q * TC:(q + 1) * TC],
                                      in_=dbl[DT_RANK:, :])

                    # --- dt = softplus(dtwT.T @ dbl[:16] + dtb) ; dtx ; y0
                    dt_bf = [one.tile([128, TC], BF16, tag=f"dt{d}", name=f"dt{d}", bufs=2) for d in range(NDT)]
                    dtx_bf = [one.tile([128, TC], BF16, tag=f"dtx{d}", name=f"dtx{d}", bufs=2) for d in range(NDT)]
                    y_acc = [one.tile([128, TC], BF16, tag=f"yac{d}", name=f"yac{d}") for d in range(NDT)]
                    for d in range(NDT):
                        for s in range(NSUB):
                            pt = ps.tile([128, SUB], F32, tag="ps")
                            nc.tensor.matmul(pt[:], dtw_t[:, d * 128:(d + 1) * 128],
                                             dbl[:DT_RANK, s * SUB:(s + 1) * SUB],
                                             start=True, stop=True)
                            et = sm.tile([128, SUB], F32, tag="spexp")
                            nc.scalar.activation(et[:], pt[:], AF.Exp,
                                                 bias=dtb_t[:, d:d + 1])
                            nc.scalar.activation(dt_bf[d][:, s * SUB:(s + 1) * SUB], et[:],
                                                 AF.Ln, bias=1.0)
                        nc.vector.tensor_tensor(dtx_bf[d][:], dt_bf[d][:], xs_bf[d][:], OP.mult)
                        nc.scalar.activation(y_acc[d][:], xs_bf[d][:], AF.Copy,
                                             scale=dsk_t[:, d:d + 1])
                    # PSUM y accumulators for d-tiles 0,1: seed with y0 via identity matmul
                    ypsum = {}
                    for d in range(2):
                        for s in range(NSUB):
                            pyt = psy.tile([128, SUB], F32, tag="psy", name=f"psy{d}_{s}")
                            ypsum[(d, s)] = pyt
                            nc.tensor.matmul(pyt[:], idb_t[:],
                                             y_acc[d][:, s * SUB:(s + 1) * SUB],
                                             start=True, stop=False,
                                             skip_group_check=True)

                    if debug_probe and li == 0 and q == 0:
                        nc.sync.dma_start(out=dbg_dt[:], in_=dt_bf[0][:])
                    # --- scan over (n, d)
                    for n in range(D_STATE):
                        B_rep = sp.tile([128, TC], BF16, tag="Brep", bufs=2)
                        nc.sync.dma_start(out=B_rep[:],
                                          in_=bc_stage[n, q * TC:(q + 1) * TC].partition_broadcast(128))
                        C_rep = sp.tile([128, TC], BF16, tag="Crep", bufs=2)
                        nc.sync.dma_start(out=C_rep[:],
                                          in_=bc_stage[D_STATE + n, q * TC:(q + 1) * TC].partition_broadcast(128))
                        for d in range(NDT):
                            a_t = sp.tile([128, TC], F32, tag="a", bufs=2)
                            nc.scalar.activation(a_t[:], dt_bf[d][:], AF.Exp,
                                                 scale=A_t[:, d * D_STATE + n: d * D_STATE + n + 1])
                            b_t = sp.tile([128, TC], BF16, tag="b", bufs=2)
                            nc.vector.tensor_tensor(b_t[:], dtx_bf[d][:], B_rep[:], OP.mult)
                            h_t = sp.tile([128, TC], BF16, tag="h")
                            init = 0.0 if q == 0 else carry[:, n * NDT + d: n * NDT + d + 1]
                            nc.vector.tensor_tensor_scan(h_t[:], a_t[:], b_t[:], init,
                                                         OP.mult, OP.add)
                            if q < NH - 1:
                                nc.vector.tensor_copy(carry[:, n * NDT + d: n * NDT + d + 1],
                                                      h_t[:, TC - 1:TC])
                            yn = sp.tile([128, TC], BF16, tag="yn", bufs=2)
                            nc.vector.tensor_tensor(yn[:], h_t[:], C_rep[:], OP.mult)
                            if d < 2:
                                for s in range(NSUB):
                                    nc.tensor.matmul(ypsum[(d, s)][:], idb_t[:],
                                                     yn[:, s * SUB:(s + 1) * SUB],
                                                     start=False,
                                                     stop=(n == D_STATE - 1),
                                                     skip_group_check=True)
                            else:
                                nc.vector.tensor_tensor(y_acc[d][:], y_acc[d][:], yn[:], OP.add)

                    for d in range(2):
                        for s in range(NSUB):
                            nc.vector.tensor_copy(y_acc[d][:, s * SUB:(s + 1) * SUB],
                                                  ypsum[(d, s)][:])
                    if debug_probe and li == 0 and q == 0:
                        nc.sync.dma_start(out=dbg_y[:], in_=y_acc[0][:])
                    # --- gate + out_proj + residual update
                    for d in range(NDT):
                        nc.vector.tensor_scalar(g_bf[d][:], g_bf[d][:], 0.5, 0.5, OP.mult, OP.add)
                        nc.vector.tensor_tensor(g_bf[d][:], g_bf[d][:], z_bf[d][:], OP.mult)
                        nc.vector.tensor_tensor(y_acc[d][:], y_acc[d][:], g_bf[d][:], OP.mult)
                    for cc in range(N128):
                        pt = ps.tile([128, D_MODEL], F32, tag="ps")
                        for d in range(NDT):
                            nc.tensor.matmul(pt[:], y_acc[d][:, cc * 128:(cc + 1) * 128],
                                             ow_t[:, d * D_MODEL:(d + 1) * D_MODEL],
                                             start=(d == 0), stop=(d == NDT - 1))
                        c = q * N128 + cc
                        nc.vector.tensor_tensor(h_res[:, c * D_MODEL:(c + 1) * D_MODEL],
                                                h_res[:, c * D_MODEL:(c + 1) * D_MODEL],
                                                pt[:], OP.add)

                if debug_probe:
                    nc.sync.dma_start(out=dbg_l[li][:], in_=h_res[:])

            # ---- final LN -> out
            nfw_t = cpool.tile([128, D_MODEL], F32)
            nfb_t = cpool.tile([128, D_MODEL], F32)
            nc.sync.dma_start(out=nfw_t[:], in_=nfw[:].partition_broadcast(128))
            nc.sync.dma_start(out=nfb_t[:], in_=nfb[:].partition_broadcast(128))

            def fdst(c, lt):
                nc.sync.dma_start(out=out_y[c * 128:(c + 1) * 128, :], in_=lt[:])
            layernorm(None, nfw_t, nfb_t, ln_affine, fdst)

    return nc


def _prep_inputs(inputs, L):
    I = {k: np.asarray(v) for k, v in inputs.items()}
    f32 = np.float32
    bf16 = ml_dtypes.bfloat16
    W = {}
    W["ident"] = np.eye(128, dtype=f32)
    W["ident_bf"] = np.eye(128, dtype=bf16)
    W["ones1"] = np.ones((1, 128), f32)
    W["emb_wT"] = np.ascontiguousarray(I["emb_w"].T).astype(f32)
    W["emb_b1"] = I["emb_b"].reshape(1, -1).astype(f32)
    W["in_wT"] = np.ascontiguousarray(
        I["in_proj_w"].transpose(0, 2, 1).reshape(N_LAYER, KDM, 128, 2 * D_INNER)).astype(f32)
    W["conv_w"] = I["conv_w"].reshape(N_LAYER, NDT, 128, D_CONV).astype(f32)
    cd = np.zeros((N_LAYER, NDT, D_CONV, 128, 128), bf16)
    cwr = I["conv_w"].reshape(N_LAYER, NDT, 128, D_CONV)
    idx = np.arange(128)
    for li_ in range(N_LAYER):
        for d_ in range(NDT):
            for k_ in range(D_CONV):
                cd[li_, d_, k_, idx, idx] = cwr[li_, d_, :, k_].astype(bf16)
    W["conv_diag"] = cd
    W["conv_b"] = I["conv_b"].reshape(N_LAYER, NDT, 128, 1).astype(f32)
    W["xpwT"] = np.ascontiguousarray(
        I["x_proj_w"].transpose(0, 2, 1).reshape(N_LAYER, NDT, 128, -1)).astype(bf16)
    W["dtwT"] = np.ascontiguousarray(I["dt_proj_w"].transpose(0, 2, 1)).astype(bf16)
    W["dtb"] = I["dt_proj_b"].reshape(N_LAYER, NDT, 128, 1).astype(f32)
    W["A_in"] = (-np.exp(I["A_log"].astype(np.float64))).reshape(
        N_LAYER, NDT, 128, D_STATE).astype(f32)
    W["Dsk"] = I["D"].reshape(N_LAYER, NDT, 128, 1).astype(f32)
    W["owT"] = np.ascontiguousarray(
        I["out_proj_w"].transpose(0, 2, 1).reshape(N_LAYER, NDT, 128, D_MODEL)).astype(bf16)
    W["nw"] = I["norm_w"].reshape(N_LAYER, 1, D_MODEL).astype(f32)
    W["nb"] = I["norm_b"].reshape(N_LAYER, 1, D_MODEL).astype(f32)
    W["nfw"] = I["normf_w"].reshape(1, D_MODEL).astype(f32)
    W["nfb"] = I["normf_b"].reshape(1, D_MODEL).astype(f32)
    ln_affine = not (
        np.all(I["norm_w"] == 1) and np.all(I["norm_b"] == 0)
        and np.all(I["normf_w"] == 1) and np.all(I["normf_b"] == 0))
    return I, W, ln_affine


LAST_EXEC_TIME_NS = None
_BOOTED = False


def _ensure_boot():
    """The axon sitecustomize boot can fail (numpy not importable that early),
    leaving the PJRT plugin auto-registered without the bass compile hook.
    Re-run boot() before jax initializes; harmless if already booted."""
    global _BOOTED
    if _BOOTED:
        return
    _BOOTED = True
    try:
        if "TRN_TERMINAL_PRECOMPUTED_JSON" in os.environ:
            from trn_agent_boot.trn_boot import boot
            boot(os.environ["TRN_TERMINAL_PRECOMPUTED_JSON"],
                 "/opt/axon/libaxon_pjrt.so")
    except Exception:
        pass


_ensure_boot()


def kernel(**inputs):
    """Entry point: full inputs in, full (B, L, D_MODEL) output back.
    If the device run fails in-process (e.g. jax was initialized before the
    axon boot fix could run), retry once in a clean subprocess."""
    try:
        return _kernel_impl(**inputs)
    except Exception:
        import traceback
        traceback.print_exc()
        return _kernel_subprocess(**inputs)


def _kernel_subprocess(**inputs):
    import subprocess, tempfile
    d = tempfile.mkdtemp(prefix="bassk_")
    inp = os.path.join(d, "in.npz")
    outp = os.path.join(d, "out.npy")
    np.savez(inp, **{k: np.asarray(v) for k, v in inputs.items()})
    here = os.path.dirname(os.path.abspath(__file__))
    driver = (
        "import sys, numpy as np\n"
        f"sys.path.insert(0, {here!r})\n"
        "import kernel\n"
        f"I = dict(np.load({inp!r}))\n"
        "out = kernel._kernel_impl(**I)\n"
        f"np.save({outp!r}, out)\n"
    )
    subprocess.run([sys.executable, "-c", driver], check=True)
    return np.load(outp)


def _kernel_impl(**inputs):
    global LAST_EXEC_TIME_NS
    _ensure_boot()
    from concourse.bass_utils import run_bass_kernel_spmd
    I, W, ln_affine = _prep_inputs(inputs, L_)
    nc = build_nc(L=L_, TC=1024, ln_affine=ln_affine)
    nc.finalize()
    core_ids = list(range(B_))
    in_maps = []
    for b in range(B_):
        m = dict(W)
        m["x_in"] = np.ascontiguousarray(I["x"][b]).astype(np.float32)
        in_maps.append(m)
    res = run_bass_kernel_spmd(nc, in_maps, core_ids)
    LAST_EXEC_TIME_NS = getattr(res, "exec_time_ns", None)
    out = np.stack([np.asarray(res.results[b]["out_y"]) for b in range(B_)])
    return out.astype(np.float32)


if __name__ == "__main__":
    # tiny CoreSim check at reduced L
    from concourse import bass_interp
    Ls = 512
    rng = np.random.default_rng(0)
    fake = {
        "x": rng.standard_normal((1, C_IN, Ls)).astype(np.float32),
        "emb_w": rng.standard_normal((D_MODEL, C_IN)).astype(np.float32) * 0.1,
        "emb_b": rng.standard_normal((D_MODEL,)).astype(np.float32) * 0.01,
        "in_proj_w": rng.standard_normal((N_LAYER, 2 * D_INNER, D_MODEL)).astype(np.float32) * 0.02,
        "conv_w": rng.standard_normal((N_LAYER, D_INNER, D_CONV)).astype(np.float32) * 0.1,
        "conv_b": rng.standard_normal((N_LAYER, D_INNER)).astype(np.float32) * 0.01,
        "x_proj_w": rng.standard_normal((N_LAYER, DT_RANK + 2 * D_STATE, D_INNER)).astype(np.float32) * 0.02,
        "dt_proj_w": rng.standard_normal((N_LAYER, D_INNER, DT_RANK)).astype(np.float32) * 0.1,
        "dt_proj_b": np.full((N_LAYER, D_INNER), -4.6, np.float32),
        "A_log": np.tile(np.log(np.arange(1, D_STATE + 1, dtype=np.float32))[None, None, :],
                          (N_LAYER, D_INNER, 1)),
        "D": np.ones((N_LAYER, D_INNER), np.float32),
        "out_proj_w": rng.standard_normal((N_LAYER, D_MODEL, D_INNER)).astype(np.float32) * 0.02,
        "norm_w": np.ones((N_LAYER, D_MODEL), np.float32),
        "norm_b": np.zeros((N_LAYER, D_MODEL), np.float32),
        "normf_w": np.ones((D_MODEL,), np.float32),
        "normf_b": np.zeros((D_MODEL,), np.float32),
    }
    I, W, ln_affine = _prep_inputs(fake, Ls)
    nc = build_nc(L=Ls, TC=256, ln_affine=ln_affine)
    sim = bass_interp.CoreSim(nc)
    for k, v in W.items():
        sim.tensor(k)[:] = v
    sim.tensor("x_in")[:] = fake["x"][0]
    sim.simulate()
    got = sim.tensor("out_y").copy()
    from mock import np_reference
    exp = np_reference(fake)[0]
    err = np.abs(got - exp)
    print("sim abs max err:", err.max(), "rel:", err.max() / np.abs(exp).max())


# revision 27
# speedup vs baseline: 1.0402x; 1.0402x over previous
"""Mamba MixerModel Trainium2 kernel.

Sharding: data-parallel over batch (8 cores x 1 batch element). No collectives.
Layout: d_inner on partitions for conv/scan; tokens on partitions for LN/residual.
Scan: native DVE tensor_tensor_scan (state = a*state + b) per (n, d-tile) strip,
with fp32 decays a = exp(A*dt) fused on ScalarE (per-partition scale), bf16
injections/outputs, sequence processed in quarters with bf16 carries.
"""
import sys, os
sys.path.insert(0, "/opt/trn_rl_repo")

import numpy as np
import ml_dtypes

import concourse.bass as bass
import concourse.bacc as bacc
import concourse.mybir as mybir
from concourse.tile import TileContext

F32 = mybir.dt.float32
BF16 = mybir.dt.bfloat16
AF = mybir.ActivationFunctionType
OP = mybir.AluOpType

B_, L_, C_IN, D_MODEL, N_LAYER = 8, 4096, 4, 256, 4
D_INNER, D_STATE, D_CONV, DT_RANK = 512, 16, 4, 16
EPS = 1e-5
NDT = D_INNER // 128  # 4 d-tiles
NMT = 2 * D_INNER // 128  # 8 xz row tiles
KDM = D_MODEL // 128  # 2 k-tiles over d_model


def build_nc(L=4096, TC=1024, ln_affine=True, debug_probe=False):
    NH = L // TC           # quarters
    NSUB = TC // 512 if TC >= 512 else 1   # psum subchunks per quarter
    SUB = min(512, TC)
    N128 = TC // 128       # 128-token chunks per quarter
    NCH = L // 128         # total 128-token chunks

    nc = bacc.Bacc(None, target_bir_lowering=False)
    dram = {}
    def din(name, shape, dt=F32):
        dram[name] = nc.dram_tensor(name, shape, dt, kind="ExternalInput")
        return dram[name]

    x_in = din("x_in", [C_IN, L])
    ident = din("ident", [128, 128])
    ident_bf = din("ident_bf", [128, 128], BF16)
    ones1 = din("ones1", [1, 128])
    emb_wT = din("emb_wT", [C_IN, D_MODEL])
    emb_b1 = din("emb_b1", [1, D_MODEL])
    in_wT = din("in_wT", [N_LAYER, KDM, 128, 2 * D_INNER])
    conv_w = din("conv_w", [N_LAYER, NDT, 128, D_CONV])
    conv_diag = din("conv_diag", [N_LAYER, NDT, D_CONV, 128, 128], BF16)
    conv_b = din("conv_b", [N_LAYER, NDT, 128, 1])
    xpwT = din("xpwT", [N_LAYER, NDT, 128, DT_RANK + 2 * D_STATE], BF16)
    dtwT = din("dtwT", [N_LAYER, DT_RANK, D_INNER], BF16)
    dtb = din("dtb", [N_LAYER, NDT, 128, 1])
    A_in = din("A_in", [N_LAYER, NDT, 128, D_STATE])
    Dsk = din("Dsk", [N_LAYER, NDT, 128, 1])
    owT = din("owT", [N_LAYER, NDT, 128, D_MODEL], BF16)
    nw = din("nw", [N_LAYER, 1, D_MODEL])
    nb = din("nb", [N_LAYER, 1, D_MODEL])
    nfw = din("nfw", [1, D_MODEL])
    nfb = din("nfb", [1, D_MODEL])
    out_y = nc.dram_tensor("out_y", [L, D_MODEL], F32, kind="ExternalOutput")
    if debug_probe:
        dbg_emb = nc.dram_tensor("dbg_emb", [128, (L // 128) * D_MODEL], F32, kind="ExternalOutput")
        dbg_hT = nc.dram_tensor("dbg_hT", [128, 2 * TC], F32, kind="ExternalOutput")
        dbg_xs = nc.dram_tensor("dbg_xs", [128, TC], BF16, kind="ExternalOutput")
        dbg_dt = nc.dram_tensor("dbg_dt", [128, TC], BF16, kind="ExternalOutput")
        dbg_dbl = nc.dram_tensor("dbg_dbl", [48, TC], BF16, kind="ExternalOutput")
        dbg_y = nc.dram_tensor("dbg_y", [128, TC], BF16, kind="ExternalOutput")
        dbg_l = [nc.dram_tensor(f"dbg_l{i}", [128, (L // 128) * D_MODEL], F32,
                                kind="ExternalOutput") for i in range(N_LAYER)]

    NX = DT_RANK + 2 * D_STATE  # 48

    with TileContext(nc) as tc:
        with (
            tc.tile_pool(name="const", bufs=1) as cpool,
            tc.tile_pool(name="wts", bufs=1) as wpool,
            tc.tile_pool(name="hres", bufs=1) as hpool,
            tc.tile_pool(name="big", bufs=2) as big,     # rotating big transients
            tc.tile_pool(name="one", bufs=1) as one,     # per-quarter single-buffered
            tc.tile_pool(name="strip", bufs=3) as sp,    # scan strips
            tc.tile_pool(name="small", bufs=2) as sm,
            tc.tile_pool(name="psum", bufs=2, space="PSUM") as ps,
            tc.tile_pool(name="psumy", bufs=6, space="PSUM") as psy,
            tc.tile_pool(name="dram", bufs=1, space="DRAM") as dpool,
        ):
            # ---- constants / global tiles
            id_t = cpool.tile([128, 128], F32)
            nc.sync.dma_start(out=id_t[:], in_=ident[:])
            idb_t = cpool.tile([128, 128], BF16)
            nc.sync.dma_start(out=idb_t[:], in_=ident_bf[:])
            ones_t = cpool.tile([1, 128], F32)
            nc.sync.dma_start(out=ones_t[:], in_=ones1[:])
            embw_t = cpool.tile([C_IN, D_MODEL], F32)
            nc.sync.dma_start(out=embw_t[:], in_=emb_wT[:])
            embb_t = cpool.tile([1, D_MODEL], F32)
            nc.sync.dma_start(out=embb_t[:], in_=emb_b1[:])
            h_res = hpool.tile([128, NCH * D_MODEL], F32)  # [t-chunk-major, dm]
            bc_stage = dpool.tile([2 * D_STATE, L], BF16)

            # ---- embedding: h_res = x @ emb_wT + emb_b
            for c in range(NCH):
                xc = sm.tile([C_IN, 128], F32, tag="xchunk")
                nc.sync.dma_start(out=xc[:], in_=x_in[:, c * 128:(c + 1) * 128])
                pt = ps.tile([128, D_MODEL], F32, tag="ps")
                nc.tensor.matmul(pt[:], xc[:], embw_t[:],
                                 start=True, stop=False)
                nc.tensor.matmul(pt[:], ones_t[:], embb_t[:], start=False, stop=True)
                nc.scalar.copy(h_res[:, c * D_MODEL:(c + 1) * D_MODEL], pt[:])
            if debug_probe:
                nc.sync.dma_start(out=dbg_emb[:], in_=h_res[:])

            def layernorm(widx, wt, bt, affine, dst_chunks):
                """LN over h_res; dst_chunks(c, tile[128, D_MODEL]) consumes output."""
                for q in range(NH):
                    c0 = q * N128
                    s1 = sm.tile([128, N128], F32, tag="st1")
                    src3 = h_res[:, c0 * D_MODEL:(c0 + N128) * D_MODEL].rearrange(
                        "p (c d) -> p c d", c=N128)
                    nc.vector.tensor_reduce(s1[:], src3, axis=mybir.AxisListType.X, op=OP.add)
                    s2 = sm.tile([128, N128], F32, tag="st2")
                    NHF = N128 // 2
                    for hh in range(2):
                        sq = one.tile([128, NHF * D_MODEL], F32, tag="sq")
                        nc.scalar.square(sq[:], h_res[:, (c0 + hh * NHF) * D_MODEL:
                                                      (c0 + (hh + 1) * NHF) * D_MODEL])
                        nc.vector.tensor_reduce(s2[:, hh * NHF:(hh + 1) * NHF],
                                                sq[:].rearrange("p (c d) -> p c d", c=NHF),
                                                axis=mybir.AxisListType.X, op=OP.add)
                    mean = sm.tile([128, N128], F32, tag="st3")
                    nc.scalar.mul(mean[:], s1[:], 1.0 / D_MODEL)
                    ex2 = sm.tile([128, N128], F32, tag="st4")
                    nc.scalar.mul(ex2[:], s2[:], 1.0 / D_MODEL)
                    var = sm.tile([128, N128], F32, tag="st5")
                    nc.vector.scalar_tensor_tensor(var[:], mean[:], -1.0, mean[:], OP.mult, OP.mult)
                    nc.vector.tensor_tensor(var[:], ex2[:], var[:], OP.add)
                    sq2 = sm.tile([128, N128], F32, tag="st6")
                    nc.vector.tensor_scalar(var[:], var[:], EPS, None, OP.add)
                    nc.scalar.activation(sq2[:], var[:], AF.Ln)
                    rstd = sm.tile([128, N128], F32, tag="st7")
                    nc.scalar.activation(rstd[:], sq2[:], AF.Exp, scale=-0.5)
                    negmr = sm.tile([128, N128], F32, tag="st8")
                    nc.vector.scalar_tensor_tensor(negmr[:], mean[:], -1.0, rstd[:], OP.mult, OP.mult)
                    for cc in range(N128):
                        c = c0 + cc
                        lt = sm.tile([128, D_MODEL], F32, tag="lnout")
                        nc.scalar.activation(lt[:], h_res[:, c * D_MODEL:(c + 1) * D_MODEL],
                                             AF.Identity, bias=negmr[:, cc:cc + 1],
                                             scale=rstd[:, cc:cc + 1])
                        if affine:
                            nc.vector.tensor_tensor(lt[:], lt[:], wt[:], OP.mult)
                            nc.vector.tensor_tensor(lt[:], lt[:], bt[:], OP.add)
                        dst_chunks(c, lt)

            # ================= layers =================
            for li in range(N_LAYER):
                # ---- load weights for this layer
                inw_t = wpool.tile([128, KDM * 2 * D_INNER], F32, tag="w_in")
                for k in range(KDM):
                    nc.sync.dma_start(out=inw_t[:, k * 2 * D_INNER:(k + 1) * 2 * D_INNER],
                                      in_=in_wT[li, k])
                cw_t = wpool.tile([128, NDT * D_CONV], F32, tag="w_cw")
                cb_t = wpool.tile([128, NDT], F32, tag="w_cb")
                xpw_t = wpool.tile([128, NDT * NX], BF16, tag="w_xp")
                dtw_t = wpool.tile([DT_RANK, D_INNER], BF16, tag="w_dtw")
                dtb_t = wpool.tile([128, NDT], F32, tag="w_dtb")
                A_t = wpool.tile([128, NDT * D_STATE], F32, tag="w_A")
                dsk_t = wpool.tile([128, NDT], F32, tag="w_dsk")
                ow_t = wpool.tile([128, NDT * D_MODEL], BF16, tag="w_ow")
                for d in range(NDT):
                    nc.sync.dma_start(out=cw_t[:, d * D_CONV:(d + 1) * D_CONV], in_=conv_w[li, d])
                    nc.sync.dma_start(out=cb_t[:, d:d + 1], in_=conv_b[li, d])
                    nc.sync.dma_start(out=xpw_t[:, d * NX:(d + 1) * NX], in_=xpwT[li, d])
                    nc.sync.dma_start(out=dtb_t[:, d:d + 1], in_=dtb[li, d])
                    nc.sync.dma_start(out=A_t[:, d * D_STATE:(d + 1) * D_STATE], in_=A_in[li, d])
                    nc.sync.dma_start(out=dsk_t[:, d:d + 1], in_=Dsk[li, d])
                    nc.sync.dma_start(out=ow_t[:, d * D_MODEL:(d + 1) * D_MODEL], in_=owT[li, d])
                nc.sync.dma_start(out=dtw_t[:], in_=dtwT[li])
                if ln_affine:
                    nw_t = wpool.tile([128, D_MODEL], F32, tag="w_nw")
                    nb_t = wpool.tile([128, D_MODEL], F32, tag="w_nb")
                    nc.sync.dma_start(out=nw_t[:], in_=nw[li].partition_broadcast(128))
                    nc.sync.dma_start(out=nb_t[:], in_=nb[li].partition_broadcast(128))
                else:
                    nw_t = nb_t = None

                carry = wpool.tile([128, D_STATE * NDT], BF16, tag="carry")
                cbh_t = wpool.tile([128, NDT], F32, tag="w_cbh")
                nc.vector.tensor_scalar(cbh_t[:], cb_t[:], 0.5, None, OP.mult)

                # per-quarter state (python lists of tiles)
                xbf_prev = [None] * NDT  # for conv halo

                # LN output -> transpose -> hT per quarter, so capture per-quarter tiles
                hT_tiles = {}

                def make_dst(hT_tiles):
                    def dst(c, lt):
                        q = c // N128
                        cc = c % N128
                        hT = hT_tiles.get(q)
                        if hT is None:
                            hT = big.tile([128, KDM * TC], F32, tag="hT")
                            hT_tiles[q] = hT
                        for km in range(KDM):
                            pt = ps.tile([128, 128], F32, tag="ps")
                            nc.tensor.transpose(pt[:], lt[:, km * 128:(km + 1) * 128], id_t[:])
                            nc.scalar.copy(hT[:, km * TC + cc * 128: km * TC + (cc + 1) * 128], pt[:])
                    return dst

                # run LN for whole sequence quarter by quarter, but we need hT per
                # quarter before the rest of that quarter's pipeline. Do it inline:
                # process quarters fully sequentially below instead.
                for q in range(NH):
                    # --- LN for this quarter's chunks + transpose into hT
                    hT_tiles_q = {}
                    dstf = make_dst(hT_tiles_q)
                    c0 = q * N128
                    # (inline layernorm for quarter q)
                    s1 = sm.tile([128, N128], F32, tag="st1")
                    src3 = h_res[:, c0 * D_MODEL:(c0 + N128) * D_MODEL].rearrange(
                        "p (c d) -> p c d", c=N128)
                    nc.vector.tensor_reduce(s1[:], src3, axis=mybir.AxisListType.X, op=OP.add)
                    s2 = sm.tile([128, N128], F32, tag="st2")
                    NHF = N128 // 2
                    for hh in range(2):
                        sq = one.tile([128, NHF * D_MODEL], F32, tag="sq")
                        nc.scalar.square(sq[:], h_res[:, (c0 + hh * NHF) * D_MODEL:
                                                      (c0 + (hh + 1) * NHF) * D_MODEL])
                        nc.vector.tensor_reduce(s2[:, hh * NHF:(hh + 1) * NHF],
                                                sq[:].rearrange("p (c d) -> p c d", c=NHF),
                                                axis=mybir.AxisListType.X, op=OP.add)
                    mean = sm.tile([128, N128], F32, tag="st3")
                    nc.scalar.mul(mean[:], s1[:], 1.0 / D_MODEL)
                    ex2 = sm.tile([128, N128], F32, tag="st4")
                    nc.scalar.mul(ex2[:], s2[:], 1.0 / D_MODEL)
                    var = sm.tile([128, N128], F32, tag="st5")
                    nc.vector.scalar_tensor_tensor(var[:], mean[:], -1.0, mean[:], OP.mult, OP.mult)
                    nc.vector.tensor_tensor(var[:], ex2[:], var[:], OP.add)
                    sq2 = sm.tile([128, N128], F32, tag="st6")
                    nc.vector.tensor_scalar(var[:], var[:], EPS, None, OP.add)
                    nc.scalar.activation(sq2[:], var[:], AF.Ln)
                    rstd = sm.tile([128, N128], F32, tag="st7")
                    nc.scalar.activation(rstd[:], sq2[:], AF.Exp, scale=-0.5)
                    negmr = sm.tile([128, N128], F32, tag="st8")
                    nc.vector.scalar_tensor_tensor(negmr[:], mean[:], -1.0, rstd[:], OP.mult, OP.mult)
                    for cc in range(N128):
                        c = c0 + cc
                        lt = sm.tile([128, D_MODEL], F32, tag="lnout")
                        nc.scalar.activation(lt[:], h_res[:, c * D_MODEL:(c + 1) * D_MODEL],
                                             AF.Identity, bias=negmr[:, cc:cc + 1],
                                             scale=rstd[:, cc:cc + 1])
                        if ln_affine:
                            nc.vector.tensor_tensor(lt[:], lt[:], nw_t[:], OP.mult)
                            nc.vector.tensor_tensor(lt[:], lt[:], nb_t[:], OP.add)
                        dstf(c, lt)
                    hT = hT_tiles_q[q]

                    if debug_probe and li == 0 and q == 0:
                        nc.sync.dma_start(out=dbg_hT[:], in_=hT[:])
                    # --- in_proj -> x (bf16, per d-tile) and g = silu(z)
                    x_bf = [big.tile([128, TC], BF16, tag=f"xbf{d}", name=f"xbf{d}") for d in range(NDT)]
                    g_bf = [one.tile([128, TC], BF16, tag=f"gbf{d}", name=f"gbf{d}") for d in range(NDT)]
                    z_bf = [one.tile([128, TC], BF16, tag=f"zbf{d}", name=f"zbf{d}") for d in range(NDT)]
                    for m in range(NMT):  # 8 row-tiles of xz
                        for s in range(NSUB):
                            pt = ps.tile([128, SUB], F32, tag="ps")
                            for k in range(KDM):
                                nc.tensor.matmul(
                                    pt[:],
                                    inw_t[:, k * 2 * D_INNER + m * 128:
                                          k * 2 * D_INNER + (m + 1) * 128],
                                    hT[:, k * TC + s * SUB: k * TC + (s + 1) * SUB],
                                    start=(k == 0), stop=(k == KDM - 1))
                            if m < NDT:
                                nc.scalar.copy(x_bf[m][:, s * SUB:(s + 1) * SUB], pt[:])
                            else:
                                d = m - NDT
                                nc.scalar.copy(z_bf[d][:, s * SUB:(s + 1) * SUB], pt[:])
                                nc.scalar.activation(g_bf[d][:, s * SUB:(s + 1) * SUB],
                                                     pt[:], AF.Tanh, scale=0.5)

                    # --- conv + silu -> xs ; dtx ; y0
                    xs_bf = [one.tile([128, TC], BF16, tag=f"xsbf{d}", name=f"xsbf{d}") for d in range(NDT)]
                    for d in range(NDT):
                        acc = big.tile([128, TC], F32, tag="cacc")
                        nc.vector.tensor_scalar(acc[:], x_bf[d][:], cw_t[:, d * D_CONV + 3:
                                                                         d * D_CONV + 4],
                                                None, OP.mult)
                        for k in range(3):  # taps with shift sh = 3-k -> weight col k
                            sh = 3 - k
                            wcol = cw_t[:, d * D_CONV + k: d * D_CONV + k + 1]
                            nc.vector.scalar_tensor_tensor(
                                acc[:, sh:TC], x_bf[d][:, 0:TC - sh], wcol,
                                acc[:, sh:TC], OP.mult, OP.add)
                            if q > 0:
                                nc.vector.scalar_tensor_tensor(
                                    acc[:, 0:sh], xbf_prev[d][:, TC - sh:TC], wcol,
                                    acc[:, 0:sh], OP.mult, OP.add)
                        th = big.tile([128, TC], BF16, tag="thbuf")
                        nc.scalar.activation(th[:], acc[:], AF.Tanh, scale=0.5,
                                             bias=cbh_t[:, d:d + 1])
                        nc.vector.tensor_scalar(th[:], th[:], 0.5, 0.5, OP.mult, OP.add)
                        ub = big.tile([128, TC], BF16, tag="ubuf")
                        nc.scalar.activation(ub[:], acc[:], AF.Identity,
                                             bias=cb_t[:, d:d + 1])
                        nc.vector.tensor_tensor(xs_bf[d][:], ub[:], th[:], OP.mult)
                    xbf_prev = x_bf
                    if debug_probe and li == 0 and q == 0:
                        nc.sync.dma_start(out=dbg_xs[:], in_=xs_bf[0][:])

                    # --- x_proj -> dbl (48 rows)
                    dbl = one.tile([NX, TC], BF16, tag="dbl")
                    for s in range(NSUB):
                        pt = ps.tile([NX, SUB], F32, tag="ps")
                        for d in range(NDT):
                            nc.tensor.matmul(pt[:], xpw_t[:, d * NX:(d + 1) * NX],
                                             xs_bf[d][:, s * SUB:(s + 1) * SUB],
                                             start=(d == 0), stop=(d == NDT - 1))
                        nc.scalar.copy(dbl[:, s * SUB:(s + 1) * SUB], pt[:])
                    if debug_probe and li == 0 and q == 0:
                        nc.sync.dma_start(out=dbg_dbl[:], in_=dbl[:])
                    # stage B/C rows to DRAM for partition-broadcast
                    nc.sync.dma_start(out=bc_stage[:, q * TC:(q + 1) * TC],
                                      in_=dbl[DT_RANK:, :])

                    # --- dt = softplus(dtwT.T @ dbl[:16] + dtb) ; dtx ; y0
                    dt_bf = [one.tile([128, TC], BF16, tag=f"dt{d}", name=f"dt{d}", bufs=2) for d in range(NDT)]
                    dtx_bf = [one.tile([128, TC], BF16, tag=f"dtx{d}", name=f"dtx{d}", bufs=2) for d in range(NDT)]
                    y_acc = [one.tile([128, TC], BF16, tag=f"yac{d}", name=f"yac{d}") for d in range(NDT)]
                    for d in range(NDT):
                        for s in range(NSUB):
                            pt = ps.tile([128, SUB], F32, tag="ps")
                            nc.tensor.matmul(pt[:], dtw_t[:, d * 128:(d + 1) * 128],
                                             dbl[:DT_RANK, s * SUB:(s + 1) * SUB],
                                             start=True, stop=True)
                            et = sm.tile([128, SUB], F32, tag="spexp")
                            nc.scalar.activation(et[:], pt[:], AF.Exp,
                                                 bias=dtb_t[:, d:d + 1])
                            nc.scalar.activation(dt_bf[d][:, s * SUB:(s + 1) * SUB], et[:],
                                                 AF.Ln, bias=1.0)
                        nc.vector.tensor_tensor(dtx_bf[d][:], dt_bf[d][:], xs_bf[d][:], OP.mult)
                        nc.scalar.activation(y_acc[d][:], xs_bf[d][:], AF.Copy,
                                             scale=dsk_t[:, d:d + 1])
                    # PSUM y accumulators for d-tiles 0,1: seed with y0 via identity matmul
                    ypsum = {}
                    for d in range(3):
                        for s in range(NSUB):
                            pyt = psy.tile([128, SUB], F32, tag="psy", name=f"psy{d}_{s}")
                            ypsum[(d, s)] = pyt
                            nc.tensor.matmul(pyt[:], idb_t[:],
                                             y_acc[d][:, s * SUB:(s + 1) * SUB],
                                             start=True, stop=False,
                                             skip_group_check=True)

                    if debug_probe and li == 0 and q == 0:
                        nc.sync.dma_start(out=dbg_dt[:], in_=dt_bf[0][:])
                    # --- scan over (n, d)
                    for n in range(D_STATE):
                        B_rep = sp.tile([128, TC], BF16, tag="Brep", bufs=2)
                        nc.sync.dma_start(out=B_rep[:],
                                          in_=bc_stage[n, q * TC:(q + 1) * TC].partition_broadcast(128))
                        C_rep = sp.tile([128, TC], BF16, tag="Crep", bufs=2)
                        nc.sync.dma_start(out=C_rep[:],
                                          in_=bc_stage[D_STATE + n, q * TC:(q + 1) * TC].partition_broadcast(128))
                        for d in range(NDT):
                            a_t = sp.tile([128, TC], F32, tag="a", bufs=2)
                            nc.scalar.activation(a_t[:], dt_bf[d][:], AF.Exp,
                                                 scale=A_t[:, d * D_STATE + n: d * D_STATE + n + 1])
                            b_t = sp.tile([128, TC], BF16, tag="b", bufs=2)
                            nc.vector.tensor_tensor(b_t[:], dtx_bf[d][:], B_rep[:], OP.mult)
                            h_t = sp.tile([128, TC], BF16, tag="h")
                            init = 0.0 if q == 0 else carry[:, n * NDT + d: n * NDT + d + 1]
                            nc.vector.tensor_tensor_scan(h_t[:], a_t[:], b_t[:], init,
                                                         OP.mult, OP.add)
                            if q < NH - 1:
                                nc.vector.tensor_copy(carry[:, n * NDT + d: n * NDT + d + 1],
                                                      h_t[:, TC - 1:TC])
                            yn = sp.tile([128, TC], BF16, tag="yn", bufs=2)
                            nc.vector.tensor_tensor(yn[:], h_t[:], C_rep[:], OP.mult)
                            if d < 3:
                                for s in range(NSUB):
                                    nc.tensor.matmul(ypsum[(d, s)][:], idb_t[:],
                                                     yn[:, s * SUB:(s + 1) * SUB],
                                                     start=False,
                                                     stop=(n == D_STATE - 1),
                                                     skip_group_check=True)
                            else:
                                nc.vector.tensor_tensor(y_acc[d][:], y_acc[d][:], yn[:], OP.add)

                    for d in range(3):
                        for s in range(NSUB):
                            nc.vector.tensor_copy(y_acc[d][:, s * SUB:(s + 1) * SUB],
                                                  ypsum[(d, s)][:])
                    if debug_probe and li == 0 and q == 0:
                        nc.sync.dma_start(out=dbg_y[:], in_=y_acc[0][:])
                    # --- gate + out_proj + residual update
                    for d in range(NDT):
                        nc.vector.tensor_scalar(g_bf[d][:], g_bf[d][:], 0.5, 0.5, OP.mult, OP.add)
                        nc.vector.tensor_tensor(g_bf[d][:], g_bf[d][:], z_bf[d][:], OP.mult)
                        nc.vector.tensor_tensor(y_acc[d][:], y_acc[d][:], g_bf[d][:], OP.mult)
                    for cc in range(N128):
                        pt = ps.tile([128, D_MODEL], F32, tag="ps")
                        for d in range(NDT):
                            nc.tensor.matmul(pt[:], y_acc[d][:, cc * 128:(cc + 1) * 128],
                                             ow_t[:, d * D_MODEL:(d + 1) * D_MODEL],
                                             start=(d == 0), stop=(d == NDT - 1))
                        c = q * N128 + cc
                        nc.vector.tensor_tensor(h_res[:, c * D_MODEL:(c + 1) * D_MODEL],
                                                h_res[:, c * D_MODEL:(c + 1) * D_MODEL],
                                                pt[:], OP.add)

                if debug_probe:
                    nc.sync.dma_start(out=dbg_l[li][:], in_=h_res[:])

            # ---- final LN -> out
            nfw_t = cpool.tile([128, D_MODEL], F32)
            nfb_t = cpool.tile([128, D_MODEL], F32)
            nc.sync.dma_start(out=nfw_t[:], in_=nfw[:].partition_broadcast(128))
            nc.sync.dma_start(out=nfb_t[:], in_=nfb[:].partition_broadcast(128))

            def fdst(c, lt):
                nc.sync.dma_start(out=out_y[c * 128:(c + 1) * 128, :], in_=lt[:])
            layernorm(None, nfw_t, nfb_t, ln_affine, fdst)

    return nc


def _prep_inputs(inputs, L):
    I = {k: np.asarray(v) for k, v in inputs.items()}
    f32 = np.float32
    bf16 = ml_dtypes.bfloat16
    W = {}
    W["ident"] = np.eye(128, dtype=f32)
    W["ident_bf"] = np.eye(128, dtype=bf16)
    W["ones1"] = np.ones((1, 128), f32)
    W["emb_wT"] = np.ascontiguousarray(I["emb_w"].T).astype(f32)
    W["emb_b1"] = I["emb_b"].reshape(1, -1).astype(f32)
    W["in_wT"] = np.ascontiguousarray(
        I["in_proj_w"].transpose(0, 2, 1).reshape(N_LAYER, KDM, 128, 2 * D_INNER)).astype(f32)
    W["conv_w"] = I["conv_w"].reshape(N_LAYER, NDT, 128, D_CONV).astype(f32)
    cd = np.zeros((N_LAYER, NDT, D_CONV, 128, 128), bf16)
    cwr = I["conv_w"].reshape(N_LAYER, NDT, 128, D_CONV)
    idx = np.arange(128)
    for li_ in range(N_LAYER):
        for d_ in range(NDT):
            for k_ in range(D_CONV):
                cd[li_, d_, k_, idx, idx] = cwr[li_, d_, :, k_].astype(bf16)
    W["conv_diag"] = cd
    W["conv_b"] = I["conv_b"].reshape(N_LAYER, NDT, 128, 1).astype(f32)
    W["xpwT"] = np.ascontiguousarray(
        I["x_proj_w"].transpose(0, 2, 1).reshape(N_LAYER, NDT, 128, -1)).astype(bf16)
    W["dtwT"] = np.ascontiguousarray(I["dt_proj_w"].transpose(0, 2, 1)).astype(bf16)
    W["dtb"] = I["dt_proj_b"].reshape(N_LAYER, NDT, 128, 1).astype(f32)
    W["A_in"] = (-np.exp(I["A_log"].astype(np.float64))).reshape(
        N_LAYER, NDT, 128, D_STATE).astype(f32)
    W["Dsk"] = I["D"].reshape(N_LAYER, NDT, 128, 1).astype(f32)
    W["owT"] = np.ascontiguousarray(
        I["out_proj_w"].transpose(0, 2, 1).reshape(N_LAYER, NDT, 128, D_MODEL)).astype(bf16)
    W["nw"] = I["norm_w"].reshape(N_LAYER, 1, D_MODEL).astype(f32)
    W["nb"] = I["norm_b"].reshape(N_LAYER, 1, D_MODEL).astype(f32)
    W["nfw"] = I["normf_w"].reshape(1, D_MODEL).astype(f32)
    W["nfb"] = I["normf_b"].reshape(1, D_MODEL).astype(f32)
    ln_affine = not (
        np.all(I["norm_w"] == 1) and np.all(I["norm_b"] == 0)
        and np.all(I["normf_w"] == 1) and np.all(I["normf_b"] == 0))
    return I, W, ln_affine


LAST_EXEC_TIME_NS = None
_BOOTED = False


def _ensure_boot():
    """The axon sitecustomize boot can fail (numpy not importable that early),
    leaving the PJRT plugin auto-registered without the bass compile hook.
    Re-run boot() before jax initializes; harmless if already booted."""
    global _BOOTED
    if _BOOTED:
        return
    _BOOTED = True
    try:
        if "TRN_TERMINAL_PRECOMPUTED_JSON" in os.environ:
            from trn_agent_boot.trn_boot import boot
            boot(os.environ["TRN_TERMINAL_PRECOMPUTED_JSON"],
                 "/opt/axon/libaxon_pjrt.so")
    except Exception:
        pass


_ensure_boot()


def kernel(**inputs):
    """Entry point: full inputs in, full (B, L, D_MODEL) output back.
    If the device run fails in-process (e.g. jax was initialized before the
    axon boot fix could run), retry once in a clean subprocess."""
    try:
        return _kernel_impl(**inputs)
    except Exception:
        import traceback
        traceback.print_exc()
        return _kernel_subprocess(**inputs)


def _kernel_subprocess(**inputs):
    import subprocess, tempfile
    d = tempfile.mkdtemp(prefix="bassk_")
    inp = os.path.join(d, "in.npz")
    outp = os.path.join(d, "out.npy")
    np.savez(inp, **{k: np.asarray(v) for k, v in inputs.items()})
    here = os.path.dirname(os.path.abspath(__file__))
    driver = (
        "import sys, numpy as np\n"
        f"sys.path.insert(0, {here!r})\n"
        "import kernel\n"
        f"I = dict(np.load({inp!r}))\n"
        "out = kernel._kernel_impl(**I)\n"
        f"np.save({outp!r}, out)\n"
    )
    subprocess.run([sys.executable, "-c", driver], check=True)
    return np.load(outp)


def _kernel_impl(**inputs):
    global LAST_EXEC_TIME_NS
    _ensure_boot()
    from concourse.bass_utils import run_bass_kernel_spmd
    I, W, ln_affine = _prep_inputs(inputs, L_)
    nc = build_nc(L=L_, TC=1024, ln_affine=ln_affine)
    nc.finalize()
    core_ids = list(range(B_))
    in_maps = []
    for b in range(B_):
        m = dict(W)
        m["x_in"] = np.ascontiguousarray(I["x"][b]).astype(np.float32)
        in_maps.append(m)
    res = run_bass_kernel_spmd(nc, in_maps, core_ids)
    LAST_EXEC_TIME_NS = getattr(res, "exec_time_ns", None)
    out = np.stack([np.asarray(res.results[b]["out_y"]) for b in range(B_)])
    return out.astype(np.float32)


if __name__ == "__main__":
    # tiny CoreSim check at reduced L
    from concourse import bass_interp
    Ls = 512
    rng = np.random.default_rng(0)
    fake = {
        "x": rng.standard_normal((1, C_IN, Ls)).astype(np.float32),
        "emb_w": rng.standard_normal((D_MODEL, C_IN)).astype(np.float32) * 0.1,
        "emb_b": rng.standard_normal((D_MODEL,)).astype(np.float32) * 0.01,
        "in_proj_w": rng.standard_normal((N_LAYER, 2 * D_INNER, D_MODEL)).astype(np.float32) * 0.02,
        "conv_w": rng.standard_normal((N_LAYER, D_INNER, D_CONV)).astype(np.float32) * 0.1,
        "conv_b": rng.standard_normal((N_LAYER, D_INNER)).astype(np.float32) * 0.01,
        "x_proj_w": rng.standard_normal((N_LAYER, DT_RANK + 2 * D_STATE, D_INNER)).astype(np.float32) * 0.02,
        "dt_proj_w": rng.standard_normal((N_LAYER, D_INNER, DT_RANK)).astype(np.float32) * 0.1,
        "dt_proj_b": np.full((N_LAYER, D_INNER), -4.6, np.float32),
        "A_log": np.tile(np.log(np.arange(1, D_STATE + 1, dtype=np.float32))[None, None, :],
                          (N_LAYER, D_INNER, 1)),
        "D": np.ones((N_LAYER, D_INNER), np.float32),
        "out_proj_w": rng.standard_normal((N_LAYER, D_MODEL, D_INNER)).astype(np.float32) * 0.02,
        "norm_w": np.ones((N_LAYER, D_MODEL), np.float32),
        "norm_b": np.zeros((N_LAYER, D_MODEL), np.float32),
        "normf_w": np.ones((D_MODEL,), np.float32),
        "normf_b": np.zeros((D_MODEL,), np.float32),
    }
    I, W, ln_affine = _prep_inputs(fake, Ls)
    nc = build_nc(L=Ls, TC=256, ln_affine=ln_affine)
    sim = bass_interp.CoreSim(nc)
    for k, v in W.items():
        sim.tensor(k)[:] = v
    sim.tensor("x_in")[:] = fake["x"][0]
    sim.simulate()
    got = sim.tensor("out_y").copy()
    from mock import np_reference
    exp = np_reference(fake)[0]
    err = np.abs(got - exp)
    print("sim abs max err:", err.max(), "rel:", err.max() / np.abs(exp).max())


# revision 28
# speedup vs baseline: 1.0577x; 1.0169x over previous
"""Mamba MixerModel Trainium2 kernel.

Sharding: data-parallel over batch (8 cores x 1 batch element). No collectives.
Layout: d_inner on partitions for conv/scan; tokens on partitions for LN/residual.
Scan: native DVE tensor_tensor_scan (state = a*state + b) per (n, d-tile) strip,
with fp32 decays a = exp(A*dt) fused on ScalarE (per-partition scale), bf16
injections/outputs, sequence processed in quarters with bf16 carries.
"""
import sys, os
sys.path.insert(0, "/opt/trn_rl_repo")

import numpy as np
import ml_dtypes

import concourse.bass as bass
import concourse.bacc as bacc
import concourse.mybir as mybir
from concourse.tile import TileContext

F32 = mybir.dt.float32
BF16 = mybir.dt.bfloat16
AF = mybir.ActivationFunctionType
OP = mybir.AluOpType

B_, L_, C_IN, D_MODEL, N_LAYER = 8, 4096, 4, 256, 4
D_INNER, D_STATE, D_CONV, DT_RANK = 512, 16, 4, 16
EPS = 1e-5
NDT = D_INNER // 128  # 4 d-tiles
NMT = 2 * D_INNER // 128  # 8 xz row tiles
KDM = D_MODEL // 128  # 2 k-tiles over d_model


def build_nc(L=4096, TC=1024, ln_affine=True, debug_probe=False):
    NH = L // TC           # quarters
    NSUB = TC // 512 if TC >= 512 else 1   # psum subchunks per quarter
    SUB = min(512, TC)
    N128 = TC // 128       # 128-token chunks per quarter
    NCH = L // 128         # total 128-token chunks

    nc = bacc.Bacc(None, target_bir_lowering=False)
    dram = {}
    def din(name, shape, dt=F32):
        dram[name] = nc.dram_tensor(name, shape, dt, kind="ExternalInput")
        return dram[name]

    x_in = din("x_in", [C_IN, L])
    ident = din("ident", [128, 128])
    ident_bf = din("ident_bf", [128, 128], BF16)
    ones1 = din("ones1", [1, 128])
    emb_wT = din("emb_wT", [C_IN, D_MODEL])
    emb_b1 = din("emb_b1", [1, D_MODEL])
    in_wT = din("in_wT", [N_LAYER, KDM, 128, 2 * D_INNER])
    conv_w = din("conv_w", [N_LAYER, NDT, 128, D_CONV])
    conv_diag = din("conv_diag", [N_LAYER, NDT, D_CONV, 128, 128], BF16)
    conv_b = din("conv_b", [N_LAYER, NDT, 128, 1])
    xpwT = din("xpwT", [N_LAYER, NDT, 128, DT_RANK + 2 * D_STATE], BF16)
    dtwT = din("dtwT", [N_LAYER, DT_RANK, D_INNER], BF16)
    dtb = din("dtb", [N_LAYER, NDT, 128, 1])
    A_in = din("A_in", [N_LAYER, NDT, 128, D_STATE])
    Dsk = din("Dsk", [N_LAYER, NDT, 128, 1])
    owT = din("owT", [N_LAYER, NDT, 128, D_MODEL], BF16)
    nw = din("nw", [N_LAYER, 1, D_MODEL])
    nb = din("nb", [N_LAYER, 1, D_MODEL])
    nfw = din("nfw", [1, D_MODEL])
    nfb = din("nfb", [1, D_MODEL])
    out_y = nc.dram_tensor("out_y", [L, D_MODEL], F32, kind="ExternalOutput")
    if debug_probe:
        dbg_emb = nc.dram_tensor("dbg_emb", [128, (L // 128) * D_MODEL], F32, kind="ExternalOutput")
        dbg_hT = nc.dram_tensor("dbg_hT", [128, 2 * TC], F32, kind="ExternalOutput")
        dbg_xs = nc.dram_tensor("dbg_xs", [128, TC], BF16, kind="ExternalOutput")
        dbg_dt = nc.dram_tensor("dbg_dt", [128, TC], BF16, kind="ExternalOutput")
        dbg_dbl = nc.dram_tensor("dbg_dbl", [48, TC], BF16, kind="ExternalOutput")
        dbg_y = nc.dram_tensor("dbg_y", [128, TC], BF16, kind="ExternalOutput")
        dbg_l = [nc.dram_tensor(f"dbg_l{i}", [128, (L // 128) * D_MODEL], F32,
                                kind="ExternalOutput") for i in range(N_LAYER)]

    NX = DT_RANK + 2 * D_STATE  # 48

    with TileContext(nc) as tc:
        with (
            tc.tile_pool(name="const", bufs=1) as cpool,
            tc.tile_pool(name="wts", bufs=1) as wpool,
            tc.tile_pool(name="hres", bufs=1) as hpool,
            tc.tile_pool(name="big", bufs=2) as big,     # rotating big transients
            tc.tile_pool(name="one", bufs=1) as one,     # per-quarter single-buffered
            tc.tile_pool(name="strip", bufs=3) as sp,    # scan strips
            tc.tile_pool(name="small", bufs=2) as sm,
            tc.tile_pool(name="psum", bufs=2, space="PSUM") as ps,
            tc.tile_pool(name="psumy", bufs=6, space="PSUM") as psy,
            tc.tile_pool(name="dram", bufs=1, space="DRAM") as dpool,
        ):
            # ---- constants / global tiles
            id_t = cpool.tile([128, 128], F32)
            nc.sync.dma_start(out=id_t[:], in_=ident[:])
            idb_t = cpool.tile([128, 128], BF16)
            nc.sync.dma_start(out=idb_t[:], in_=ident_bf[:])
            ones_t = cpool.tile([1, 128], F32)
            nc.sync.dma_start(out=ones_t[:], in_=ones1[:])
            embw_t = cpool.tile([C_IN, D_MODEL], F32)
            nc.sync.dma_start(out=embw_t[:], in_=emb_wT[:])
            embb_t = cpool.tile([1, D_MODEL], F32)
            nc.sync.dma_start(out=embb_t[:], in_=emb_b1[:])
            h_res = hpool.tile([128, NCH * D_MODEL], F32)  # [t-chunk-major, dm]
            bc_stage = dpool.tile([2 * D_STATE, L], BF16)

            # ---- embedding: h_res = x @ emb_wT + emb_b
            for c in range(NCH):
                xc = sm.tile([C_IN, 128], F32, tag="xchunk")
                nc.sync.dma_start(out=xc[:], in_=x_in[:, c * 128:(c + 1) * 128])
                pt = ps.tile([128, D_MODEL], F32, tag="ps")
                nc.tensor.matmul(pt[:], xc[:], embw_t[:],
                                 start=True, stop=False)
                nc.tensor.matmul(pt[:], ones_t[:], embb_t[:], start=False, stop=True)
                nc.scalar.copy(h_res[:, c * D_MODEL:(c + 1) * D_MODEL], pt[:])
            if debug_probe:
                nc.sync.dma_start(out=dbg_emb[:], in_=h_res[:])

            def layernorm(widx, wt, bt, affine, dst_chunks):
                """LN over h_res; dst_chunks(c, tile[128, D_MODEL]) consumes output."""
                for q in range(NH):
                    c0 = q * N128
                    s1 = sm.tile([128, N128], F32, tag="st1")
                    src3 = h_res[:, c0 * D_MODEL:(c0 + N128) * D_MODEL].rearrange(
                        "p (c d) -> p c d", c=N128)
                    nc.vector.tensor_reduce(s1[:], src3, axis=mybir.AxisListType.X, op=OP.add)
                    s2 = sm.tile([128, N128], F32, tag="st2")
                    NHF = N128 // 2
                    for hh in range(2):
                        sq = one.tile([128, NHF * D_MODEL], F32, tag="sq")
                        nc.scalar.square(sq[:], h_res[:, (c0 + hh * NHF) * D_MODEL:
                                                      (c0 + (hh + 1) * NHF) * D_MODEL])
                        nc.vector.tensor_reduce(s2[:, hh * NHF:(hh + 1) * NHF],
                                                sq[:].rearrange("p (c d) -> p c d", c=NHF),
                                                axis=mybir.AxisListType.X, op=OP.add)
                    mean = sm.tile([128, N128], F32, tag="st3")
                    nc.scalar.mul(mean[:], s1[:], 1.0 / D_MODEL)
                    ex2 = sm.tile([128, N128], F32, tag="st4")
                    nc.scalar.mul(ex2[:], s2[:], 1.0 / D_MODEL)
                    var = sm.tile([128, N128], F32, tag="st5")
                    nc.vector.scalar_tensor_tensor(var[:], mean[:], -1.0, mean[:], OP.mult, OP.mult)
                    nc.vector.tensor_tensor(var[:], ex2[:], var[:], OP.add)
                    sq2 = sm.tile([128, N128], F32, tag="st6")
                    nc.vector.tensor_scalar(var[:], var[:], EPS, None, OP.add)
                    nc.scalar.activation(sq2[:], var[:], AF.Ln)
                    rstd = sm.tile([128, N128], F32, tag="st7")
                    nc.scalar.activation(rstd[:], sq2[:], AF.Exp, scale=-0.5)
                    negmr = sm.tile([128, N128], F32, tag="st8")
                    nc.vector.scalar_tensor_tensor(negmr[:], mean[:], -1.0, rstd[:], OP.mult, OP.mult)
                    for cc in range(N128):
                        c = c0 + cc
                        lt = sm.tile([128, D_MODEL], F32, tag="lnout")
                        nc.scalar.activation(lt[:], h_res[:, c * D_MODEL:(c + 1) * D_MODEL],
                                             AF.Identity, bias=negmr[:, cc:cc + 1],
                                             scale=rstd[:, cc:cc + 1])
                        if affine:
                            nc.vector.tensor_tensor(lt[:], lt[:], wt[:], OP.mult)
                            nc.vector.tensor_tensor(lt[:], lt[:], bt[:], OP.add)
                        dst_chunks(c, lt)

            # ================= layers =================
            for li in range(N_LAYER):
                # ---- load weights for this layer
                inw_t = wpool.tile([128, KDM * 2 * D_INNER], F32, tag="w_in")
                for k in range(KDM):
                    nc.sync.dma_start(out=inw_t[:, k * 2 * D_INNER:(k + 1) * 2 * D_INNER],
                                      in_=in_wT[li, k])
                cw_t = wpool.tile([128, NDT * D_CONV], F32, tag="w_cw")
                cb_t = wpool.tile([128, NDT], F32, tag="w_cb")
                xpw_t = wpool.tile([128, NDT * NX], BF16, tag="w_xp")
                dtw_t = wpool.tile([DT_RANK, D_INNER], BF16, tag="w_dtw")
                dtb_t = wpool.tile([128, NDT], F32, tag="w_dtb")
                A_t = wpool.tile([128, NDT * D_STATE], F32, tag="w_A")
                dsk_t = wpool.tile([128, NDT], F32, tag="w_dsk")
                ow_t = wpool.tile([128, NDT * D_MODEL], BF16, tag="w_ow")
                for d in range(NDT):
                    nc.sync.dma_start(out=cw_t[:, d * D_CONV:(d + 1) * D_CONV], in_=conv_w[li, d])
                    nc.sync.dma_start(out=cb_t[:, d:d + 1], in_=conv_b[li, d])
                    nc.sync.dma_start(out=xpw_t[:, d * NX:(d + 1) * NX], in_=xpwT[li, d])
                    nc.sync.dma_start(out=dtb_t[:, d:d + 1], in_=dtb[li, d])
                    nc.sync.dma_start(out=A_t[:, d * D_STATE:(d + 1) * D_STATE], in_=A_in[li, d])
                    nc.sync.dma_start(out=dsk_t[:, d:d + 1], in_=Dsk[li, d])
                    nc.sync.dma_start(out=ow_t[:, d * D_MODEL:(d + 1) * D_MODEL], in_=owT[li, d])
                nc.sync.dma_start(out=dtw_t[:], in_=dtwT[li])
                if ln_affine:
                    nw_t = wpool.tile([128, D_MODEL], F32, tag="w_nw")
                    nb_t = wpool.tile([128, D_MODEL], F32, tag="w_nb")
                    nc.sync.dma_start(out=nw_t[:], in_=nw[li].partition_broadcast(128))
                    nc.sync.dma_start(out=nb_t[:], in_=nb[li].partition_broadcast(128))
                else:
                    nw_t = nb_t = None

                carry = wpool.tile([128, D_STATE * NDT], BF16, tag="carry")
                cbh_t = wpool.tile([128, NDT], F32, tag="w_cbh")
                nc.vector.tensor_scalar(cbh_t[:], cb_t[:], 0.5, None, OP.mult)

                # per-quarter state (python lists of tiles)
                xbf_prev = [None] * NDT  # for conv halo

                # LN output -> transpose -> hT per quarter, so capture per-quarter tiles
                hT_tiles = {}

                def make_dst(hT_tiles):
                    def dst(c, lt):
                        q = c // N128
                        cc = c % N128
                        hT = hT_tiles.get(q)
                        if hT is None:
                            hT = big.tile([128, KDM * TC], F32, tag="hT")
                            hT_tiles[q] = hT
                        for km in range(KDM):
                            pt = ps.tile([128, 128], F32, tag="ps")
                            nc.tensor.transpose(pt[:], lt[:, km * 128:(km + 1) * 128], id_t[:])
                            nc.scalar.copy(hT[:, km * TC + cc * 128: km * TC + (cc + 1) * 128], pt[:])
                    return dst

                # run LN for whole sequence quarter by quarter, but we need hT per
                # quarter before the rest of that quarter's pipeline. Do it inline:
                # process quarters fully sequentially below instead.
                for q in range(NH):
                    # --- LN for this quarter's chunks + transpose into hT
                    hT_tiles_q = {}
                    dstf = make_dst(hT_tiles_q)
                    c0 = q * N128
                    # (inline layernorm for quarter q)
                    s1 = sm.tile([128, N128], F32, tag="st1")
                    src3 = h_res[:, c0 * D_MODEL:(c0 + N128) * D_MODEL].rearrange(
                        "p (c d) -> p c d", c=N128)
                    nc.vector.tensor_reduce(s1[:], src3, axis=mybir.AxisListType.X, op=OP.add)
                    s2 = sm.tile([128, N128], F32, tag="st2")
                    NHF = N128 // 2
                    for hh in range(2):
                        sq = one.tile([128, NHF * D_MODEL], F32, tag="sq")
                        nc.scalar.square(sq[:], h_res[:, (c0 + hh * NHF) * D_MODEL:
                                                      (c0 + (hh + 1) * NHF) * D_MODEL])
                        nc.vector.tensor_reduce(s2[:, hh * NHF:(hh + 1) * NHF],
                                                sq[:].rearrange("p (c d) -> p c d", c=NHF),
                                                axis=mybir.AxisListType.X, op=OP.add)
                    mean = sm.tile([128, N128], F32, tag="st3")
                    nc.scalar.mul(mean[:], s1[:], 1.0 / D_MODEL)
                    ex2 = sm.tile([128, N128], F32, tag="st4")
                    nc.scalar.mul(ex2[:], s2[:], 1.0 / D_MODEL)
                    var = sm.tile([128, N128], F32, tag="st5")
                    nc.vector.scalar_tensor_tensor(var[:], mean[:], -1.0, mean[:], OP.mult, OP.mult)
                    nc.vector.tensor_tensor(var[:], ex2[:], var[:], OP.add)
                    sq2 = sm.tile([128, N128], F32, tag="st6")
                    nc.vector.tensor_scalar(var[:], var[:], EPS, None, OP.add)
                    nc.scalar.activation(sq2[:], var[:], AF.Ln)
                    rstd = sm.tile([128, N128], F32, tag="st7")
                    nc.scalar.activation(rstd[:], sq2[:], AF.Exp, scale=-0.5)
                    negmr = sm.tile([128, N128], F32, tag="st8")
                    nc.vector.scalar_tensor_tensor(negmr[:], mean[:], -1.0, rstd[:], OP.mult, OP.mult)
                    for cc in range(N128):
                        c = c0 + cc
                        lt = sm.tile([128, D_MODEL], F32, tag="lnout")
                        nc.scalar.activation(lt[:], h_res[:, c * D_MODEL:(c + 1) * D_MODEL],
                                             AF.Identity, bias=negmr[:, cc:cc + 1],
                                             scale=rstd[:, cc:cc + 1])
                        if ln_affine:
                            nc.vector.tensor_tensor(lt[:], lt[:], nw_t[:], OP.mult)
                            nc.vector.tensor_tensor(lt[:], lt[:], nb_t[:], OP.add)
                        dstf(c, lt)
                    hT = hT_tiles_q[q]

                    if debug_probe and li == 0 and q == 0:
                        nc.sync.dma_start(out=dbg_hT[:], in_=hT[:])
                    # --- in_proj -> x (bf16, per d-tile) and g = silu(z)
                    x_bf = [big.tile([128, TC], BF16, tag=f"xbf{d}", name=f"xbf{d}") for d in range(NDT)]
                    g_bf = [one.tile([128, TC], BF16, tag=f"gbf{d}", name=f"gbf{d}") for d in range(NDT)]
                    z_bf = [one.tile([128, TC], BF16, tag=f"zbf{d}", name=f"zbf{d}") for d in range(NDT)]
                    for m in range(NDT):  # x row-tiles first (feeder-critical)
                        for s in range(NSUB):
                            pt = ps.tile([128, SUB], F32, tag="ps")
                            for k in range(KDM):
                                nc.tensor.matmul(
                                    pt[:],
                                    inw_t[:, k * 2 * D_INNER + m * 128:
                                          k * 2 * D_INNER + (m + 1) * 128],
                                    hT[:, k * TC + s * SUB: k * TC + (s + 1) * SUB],
                                    start=(k == 0), stop=(k == KDM - 1))
                            nc.scalar.copy(x_bf[m][:, s * SUB:(s + 1) * SUB], pt[:])

                    # --- conv + silu -> xs ; dtx ; y0
                    xs_bf = [one.tile([128, TC], BF16, tag=f"xsbf{d}", name=f"xsbf{d}") for d in range(NDT)]
                    for d in range(NDT):
                        acc = big.tile([128, TC], F32, tag="cacc")
                        nc.vector.tensor_scalar(acc[:], x_bf[d][:], cw_t[:, d * D_CONV + 3:
                                                                         d * D_CONV + 4],
                                                None, OP.mult)
                        for k in range(3):  # taps with shift sh = 3-k -> weight col k
                            sh = 3 - k
                            wcol = cw_t[:, d * D_CONV + k: d * D_CONV + k + 1]
                            nc.vector.scalar_tensor_tensor(
                                acc[:, sh:TC], x_bf[d][:, 0:TC - sh], wcol,
                                acc[:, sh:TC], OP.mult, OP.add)
                            if q > 0:
                                nc.vector.scalar_tensor_tensor(
                                    acc[:, 0:sh], xbf_prev[d][:, TC - sh:TC], wcol,
                                    acc[:, 0:sh], OP.mult, OP.add)
                        th = big.tile([128, TC], BF16, tag="thbuf")
                        nc.scalar.activation(th[:], acc[:], AF.Tanh, scale=0.5,
                                             bias=cbh_t[:, d:d + 1])
                        nc.vector.tensor_scalar(th[:], th[:], 0.5, 0.5, OP.mult, OP.add)
                        ub = big.tile([128, TC], BF16, tag="ubuf")
                        nc.scalar.activation(ub[:], acc[:], AF.Identity,
                                             bias=cb_t[:, d:d + 1])
                        nc.vector.tensor_tensor(xs_bf[d][:], ub[:], th[:], OP.mult)
                    xbf_prev = x_bf
                    if debug_probe and li == 0 and q == 0:
                        nc.sync.dma_start(out=dbg_xs[:], in_=xs_bf[0][:])

                    # --- x_proj -> dbl (48 rows)
                    dbl = one.tile([NX, TC], BF16, tag="dbl")
                    for s in range(NSUB):
                        pt = ps.tile([NX, SUB], F32, tag="ps")
                        for d in range(NDT):
                            nc.tensor.matmul(pt[:], xpw_t[:, d * NX:(d + 1) * NX],
                                             xs_bf[d][:, s * SUB:(s + 1) * SUB],
                                             start=(d == 0), stop=(d == NDT - 1))
                        nc.scalar.copy(dbl[:, s * SUB:(s + 1) * SUB], pt[:])
                    if debug_probe and li == 0 and q == 0:
                        nc.sync.dma_start(out=dbg_dbl[:], in_=dbl[:])
                    # stage B/C rows to DRAM for partition-broadcast
                    nc.sync.dma_start(out=bc_stage[:, q * TC:(q + 1) * TC],
                                      in_=dbl[DT_RANK:, :])

                    # --- dt = softplus(dtwT.T @ dbl[:16] + dtb) ; dtx ; y0
                    dt_bf = [one.tile([128, TC], BF16, tag=f"dt{d}", name=f"dt{d}", bufs=2) for d in range(NDT)]
                    dtx_bf = [one.tile([128, TC], BF16, tag=f"dtx{d}", name=f"dtx{d}", bufs=2) for d in range(NDT)]
                    y_acc = [one.tile([128, TC], BF16, tag=f"yac{d}", name=f"yac{d}") for d in range(NDT)]
                    for d in range(NDT):
                        for s in range(NSUB):
                            pt = ps.tile([128, SUB], F32, tag="ps")
                            nc.tensor.matmul(pt[:], dtw_t[:, d * 128:(d + 1) * 128],
                                             dbl[:DT_RANK, s * SUB:(s + 1) * SUB],
                                             start=True, stop=True)
                            et = sm.tile([128, SUB], F32, tag="spexp")
                            nc.scalar.activation(et[:], pt[:], AF.Exp,
                                                 bias=dtb_t[:, d:d + 1])
                            nc.scalar.activation(dt_bf[d][:, s * SUB:(s + 1) * SUB], et[:],
                                                 AF.Ln, bias=1.0)
                        nc.vector.tensor_tensor(dtx_bf[d][:], dt_bf[d][:], xs_bf[d][:], OP.mult)
                        nc.scalar.activation(y_acc[d][:], xs_bf[d][:], AF.Copy,
                                             scale=dsk_t[:, d:d + 1])
                    # z half of in_proj deferred here: its evac tiles pin PSUM
                    # slots until the previous quarter's gate, so keep it off the
                    # dt/a feeder path.
                    for m in range(NDT, NMT):
                        for s in range(NSUB):
                            pt = ps.tile([128, SUB], F32, tag="ps")
                            for k in range(KDM):
                                nc.tensor.matmul(
                                    pt[:],
                                    inw_t[:, k * 2 * D_INNER + m * 128:
                                          k * 2 * D_INNER + (m + 1) * 128],
                                    hT[:, k * TC + s * SUB: k * TC + (s + 1) * SUB],
                                    start=(k == 0), stop=(k == KDM - 1))
                            d = m - NDT
                            nc.scalar.copy(z_bf[d][:, s * SUB:(s + 1) * SUB], pt[:])
                            nc.scalar.activation(g_bf[d][:, s * SUB:(s + 1) * SUB],
                                                 pt[:], AF.Tanh, scale=0.5)
                    # PSUM y accumulators for d-tiles 0,1: seed with y0 via identity matmul
                    ypsum = {}
                    for d in range(3):
                        for s in range(NSUB):
                            pyt = psy.tile([128, SUB], F32, tag="psy", name=f"psy{d}_{s}")
                            ypsum[(d, s)] = pyt
                            nc.tensor.matmul(pyt[:], idb_t[:],
                                             y_acc[d][:, s * SUB:(s + 1) * SUB],
                                             start=True, stop=False,
                                             skip_group_check=True)

                    if debug_probe and li == 0 and q == 0:
                        nc.sync.dma_start(out=dbg_dt[:], in_=dt_bf[0][:])
                    # --- scan over (n, d)
                    for n in range(D_STATE):
                        B_rep = sp.tile([128, TC], BF16, tag="Brep", bufs=2)
                        nc.sync.dma_start(out=B_rep[:],
                                          in_=bc_stage[n, q * TC:(q + 1) * TC].partition_broadcast(128))
                        C_rep = sp.tile([128, TC], BF16, tag="Crep", bufs=2)
                        nc.sync.dma_start(out=C_rep[:],
                                          in_=bc_stage[D_STATE + n, q * TC:(q + 1) * TC].partition_broadcast(128))
                        for d in range(NDT):
                            a_t = sp.tile([128, TC], F32, tag="a", bufs=2)
                            nc.scalar.activation(a_t[:], dt_bf[d][:], AF.Exp,
                                                 scale=A_t[:, d * D_STATE + n: d * D_STATE + n + 1])
                            b_t = sp.tile([128, TC], BF16, tag="b", bufs=2)
                            nc.vector.tensor_tensor(b_t[:], dtx_bf[d][:], B_rep[:], OP.mult)
                            h_t = sp.tile([128, TC], BF16, tag="h")
                            init = 0.0 if q == 0 else carry[:, n * NDT + d: n * NDT + d + 1]
                            nc.vector.tensor_tensor_scan(h_t[:], a_t[:], b_t[:], init,
                                                         OP.mult, OP.add)
                            if q < NH - 1:
                                nc.vector.tensor_copy(carry[:, n * NDT + d: n * NDT + d + 1],
                                                      h_t[:, TC - 1:TC])
                            yn = sp.tile([128, TC], BF16, tag="yn", bufs=2)
                            nc.vector.tensor_tensor(yn[:], h_t[:], C_rep[:], OP.mult)
                            if d < 3:
                                for s in range(NSUB):
                                    nc.tensor.matmul(ypsum[(d, s)][:], idb_t[:],
                                                     yn[:, s * SUB:(s + 1) * SUB],
                                                     start=False,
                                                     stop=(n == D_STATE - 1),
                                                     skip_group_check=True)
                            else:
                                nc.vector.tensor_tensor(y_acc[d][:], y_acc[d][:], yn[:], OP.add)

                    for d in range(3):
                        for s in range(NSUB):
                            nc.scalar.copy(y_acc[d][:, s * SUB:(s + 1) * SUB],
                                           ypsum[(d, s)][:])
                    if debug_probe and li == 0 and q == 0:
                        nc.sync.dma_start(out=dbg_y[:], in_=y_acc[0][:])
                    # --- gate + out_proj + residual update
                    for d in range(NDT):
                        nc.vector.tensor_scalar(g_bf[d][:], g_bf[d][:], 0.5, 0.5, OP.mult, OP.add)
                        nc.vector.tensor_tensor(g_bf[d][:], g_bf[d][:], z_bf[d][:], OP.mult)
                        nc.vector.tensor_tensor(y_acc[d][:], y_acc[d][:], g_bf[d][:], OP.mult)
                    for cc in range(N128):
                        pt = ps.tile([128, D_MODEL], F32, tag="ps")
                        for d in range(NDT):
                            nc.tensor.matmul(pt[:], y_acc[d][:, cc * 128:(cc + 1) * 128],
                                             ow_t[:, d * D_MODEL:(d + 1) * D_MODEL],
                                             start=(d == 0), stop=(d == NDT - 1))
                        c = q * N128 + cc
                        nc.vector.tensor_tensor(h_res[:, c * D_MODEL:(c + 1) * D_MODEL],
                                                h_res[:, c * D_MODEL:(c + 1) * D_MODEL],
                                                pt[:], OP.add)

                if debug_probe:
                    nc.sync.dma_start(out=dbg_l[li][:], in_=h_res[:])

            # ---- final LN -> out
            nfw_t = cpool.tile([128, D_MODEL], F32)
            nfb_t = cpool.tile([128, D_MODEL], F32)
            nc.sync.dma_start(out=nfw_t[:], in_=nfw[:].partition_broadcast(128))
            nc.sync.dma_start(out=nfb_t[:], in_=nfb[:].partition_broadcast(128))

            def fdst(c, lt):
                nc.sync.dma_start(out=out_y[c * 128:(c + 1) * 128, :], in_=lt[:])
            layernorm(None, nfw_t, nfb_t, ln_affine, fdst)

    return nc


def _prep_inputs(inputs, L):
    I = {k: np.asarray(v) for k, v in inputs.items()}
    f32 = np.float32
    bf16 = ml_dtypes.bfloat16
    W = {}
    W["ident"] = np.eye(128, dtype=f32)
    W["ident_bf"] = np.eye(128, dtype=bf16)
    W["ones1"] = np.ones((1, 128), f32)
    W["emb_wT"] = np.ascontiguousarray(I["emb_w"].T).astype(f32)
    W["emb_b1"] = I["emb_b"].reshape(1, -1).astype(f32)
    W["in_wT"] = np.ascontiguousarray(
        I["in_proj_w"].transpose(0, 2, 1).reshape(N_LAYER, KDM, 128, 2 * D_INNER)).astype(f32)
    W["conv_w"] = I["conv_w"].reshape(N_LAYER, NDT, 128, D_CONV).astype(f32)
    cd = np.zeros((N_LAYER, NDT, D_CONV, 128, 128), bf16)
    cwr = I["conv_w"].reshape(N_LAYER, NDT, 128, D_CONV)
    idx = np.arange(128)
    for li_ in range(N_LAYER):
        for d_ in range(NDT):
            for k_ in range(D_CONV):
                cd[li_, d_, k_, idx, idx] = cwr[li_, d_, :, k_].astype(bf16)
    W["conv_diag"] = cd
    W["conv_b"] = I["conv_b"].reshape(N_LAYER, NDT, 128, 1).astype(f32)
    W["xpwT"] = np.ascontiguousarray(
        I["x_proj_w"].transpose(0, 2, 1).reshape(N_LAYER, NDT, 128, -1)).astype(bf16)
    W["dtwT"] = np.ascontiguousarray(I["dt_proj_w"].transpose(0, 2, 1)).astype(bf16)
    W["dtb"] = I["dt_proj_b"].reshape(N_LAYER, NDT, 128, 1).astype(f32)
    W["A_in"] = (-np.exp(I["A_log"].astype(np.float64))).reshape(
        N_LAYER, NDT, 128, D_STATE).astype(f32)
    W["Dsk"] = I["D"].reshape(N_LAYER, NDT, 128, 1).astype(f32)
    W["owT"] = np.ascontiguousarray(
        I["out_proj_w"].transpose(0, 2, 1).reshape(N_LAYER, NDT, 128, D_MODEL)).astype(bf16)
    W["nw"] = I["norm_w"].reshape(N_LAYER, 1, D_MODEL).astype(f32)
    W["nb"] = I["norm_b"].reshape(N_LAYER, 1, D_MODEL).astype(f32)
    W["nfw"] = I["normf_w"].reshape(1, D_MODEL).astype(f32)
    W["nfb"] = I["normf_b"].reshape(1, D_MODEL).astype(f32)
    ln_affine = not (
        np.all(I["norm_w"] == 1) and np.all(I["norm_b"] == 0)
        and np.all(I["normf_w"] == 1) and np.all(I["normf_b"] == 0))
    return I, W, ln_affine


LAST_EXEC_TIME_NS = None
_BOOTED = False


def _ensure_boot():
    """The axon sitecustomize boot can fail (numpy not importable that early),
    leaving the PJRT plugin auto-registered without the bass compile hook.
    Re-run boot() before jax initializes; harmless if already booted."""
    global _BOOTED
    if _BOOTED:
        return
    _BOOTED = True
    try:
        if "TRN_TERMINAL_PRECOMPUTED_JSON" in os.environ:
            from trn_agent_boot.trn_boot import boot
            boot(os.environ["TRN_TERMINAL_PRECOMPUTED_JSON"],
                 "/opt/axon/libaxon_pjrt.so")
    except Exception:
        pass


_ensure_boot()


def kernel(**inputs):
    """Entry point: full inputs in, full (B, L, D_MODEL) output back.
    If the device run fails in-process (e.g. jax was initialized before the
    axon boot fix could run), retry once in a clean subprocess."""
    try:
        return _kernel_impl(**inputs)
    except Exception:
        import traceback
        traceback.print_exc()
        return _kernel_subprocess(**inputs)


def _kernel_subprocess(**inputs):
    import subprocess, tempfile
    d = tempfile.mkdtemp(prefix="bassk_")
    inp = os.path.join(d, "in.npz")
    outp = os.path.join(d, "out.npy")
    np.savez(inp, **{k: np.asarray(v) for k, v in inputs.items()})
    here = os.path.dirname(os.path.abspath(__file__))
    driver = (
        "import sys, numpy as np\n"
        f"sys.path.insert(0, {here!r})\n"
        "import kernel\n"
        f"I = dict(np.load({inp!r}))\n"
        "out = kernel._kernel_impl(**I)\n"
        f"np.save({outp!r}, out)\n"
    )
    subprocess.run([sys.executable, "-c", driver], check=True)
    return np.load(outp)


def _kernel_impl(**inputs):
    global LAST_EXEC_TIME_NS
    _ensure_boot()
    from concourse.bass_utils import run_bass_kernel_spmd
    I, W, ln_affine = _prep_inputs(inputs, L_)
    nc = build_nc(L=L_, TC=1024, ln_affine=ln_affine)
    nc.finalize()
    core_ids = list(range(B_))
    in_maps = []
    for b in range(B_):
        m = dict(W)
        m["x_in"] = np.ascontiguousarray(I["x"][b]).astype(np.float32)
        in_maps.append(m)
    res = run_bass_kernel_spmd(nc, in_maps, core_ids)
    LAST_EXEC_TIME_NS = getattr(res, "exec_time_ns", None)
    out = np.stack([np.asarray(res.results[b]["out_y"]) for b in range(B_)])
    return out.astype(np.float32)


if __name__ == "__main__":
    # tiny CoreSim check at reduced L
    from concourse import bass_interp
    Ls = 512
    rng = np.random.default_rng(0)
    fake = {
        "x": rng.standard_normal((1, C_IN, Ls)).astype(np.float32),
        "emb_w": rng.standard_normal((D_MODEL, C_IN)).astype(np.float32) * 0.1,
        "emb_b": rng.standard_normal((D_MODEL,)).astype(np.float32) * 0.01,
        "in_proj_w": rng.standard_normal((N_LAYER, 2 * D_INNER, D_MODEL)).astype(np.float32) * 0.02,
        "conv_w": rng.standard_normal((N_LAYER, D_INNER, D_CONV)).astype(np.float32) * 0.1,
        "conv_b": rng.standard_normal((N_LAYER, D_INNER)).astype(np.float32) * 0.01,
        "x_proj_w": rng.standard_normal((N_LAYER, DT_RANK + 2 * D_STATE, D_INNER)).astype(np.float32) * 0.02,
        "dt_proj_w": rng.standard_normal((N_LAYER, D_INNER, DT_RANK)).astype(np.float32) * 0.1,
        "dt_proj_b": np.full((N_LAYER, D_INNER), -4.6, np.float32),
        "A_log": np.tile(np.log(np.arange(1, D_STATE + 1, dtype=np.float32))[None, None, :],
                          (N_LAYER, D_INNER, 1)),
        "D": np.ones((N_LAYER, D_INNER), np.float32),
        "out_proj_w": rng.standard_normal((N_LAYER, D_MODEL, D_INNER)).astype(np.float32) * 0.02,
        "norm_w": np.ones((N_LAYER, D_MODEL), np.float32),
        "norm_b": np.zeros((N_LAYER, D_MODEL), np.float32),
        "normf_w": np.ones((D_MODEL,), np.float32),
        "normf_b": np.zeros((D_MODEL,), np.float32),
    }
    I, W, ln_affine = _prep_inputs(fake, Ls)
    nc = build_nc(L=Ls, TC=256, ln_affine=ln_affine)
    sim = bass_interp.CoreSim(nc)
    for k, v in W.items():
        sim.tensor(k)[:] = v
    sim.tensor("x_in")[:] = fake["x"][0]
    sim.simulate()
    got = sim.tensor("out_y").copy()
    from mock import np_reference
    exp = np_reference(fake)[0]
    err = np.abs(got - exp)
    print("sim abs max err:", err.max(), "rel:", err.max() / np.abs(exp).max())
